# revision 10
# baseline (speedup 1.0000x reference)
"""CrossAttention Trainium2 kernel (8 NeuronCores, SPMD).

Sharding: data-parallel over batch B=2, tensor-parallel over the 16 heads in
4 groups of 4 heads -> 8 cores, one (batch, head-group) pair each. Each core
computes its 4 heads' Q/K/V projections, masked softmax cross-attention, and
its partial output projection y_g = softmax(q k^T * scale) v @ Wo[:, g].T.
The host sums the 4 partial outputs per batch (the Wo row-split all-reduce,
done at unshard time) and adds the v-bias term Wo @ b_v, which is constant
across rows and factors out of the attention (softmax rows sum to 1).

Numerics: fp16 datapath with fp32 PSUM accumulation everywhere except the
scores matmul, which runs in fp8e4m3 DoubleRow perf mode (0.5 cycles/row on
the PE, halving the dominant matmul). q and k are quantized to fp8 at their
projection evictions; the quantization error only perturbs the softmax
weights, damped by the small score scale. Everything on the V path stays
fp16 (v-side quantization would hit the output linearly).

DoubleRow layout: the PE contracts 2 fp8 values per partition; operands are
[64, 2, free] APs (matmul base partitions are restricted to {0, 32, 64}, so
heads are processed in 64-partition PAIR blocks). Per pair, kT8's two
t-slots hold the even head's and the odd head's d-values; each head's q
operand carries its own d-values in its slot and ZEROS in the partner's
slot, so the cross-head products vanish and one DR matmul yields one head's
scores at half the bf16 cost. wq and wk are column-permuted ON THE HOST
(psum block a = heads with parity a, partition hp*64+d) so the q/k
evictions are plain partition-aligned casts.

Layout: activations and weights arrive contraction-major (pre-transposed on
the host) so every device DMA is a contiguous row load. Scores are computed
transposed: ST[m, n] per head, so the PV matmul contracts over m directly,
and an appended ones-column on the V stationary operand yields the softmax
denominator for free. exp() is unnormalized (no max subtraction; scores are
bounded); mask zeros are applied multiplicatively after exp (DVE, 2x mode).

Normalization happens straight out of the PV PSUM accumulators (no fp32
park): denominator row 64 -> partition 0 via a tiny SBUF shift DMA,
reciprocal (DVE), partition_broadcast (Pool), then one fused
multiply-evict per head writes normalized fp16 into otn2. Odd heads reach
partitions 64:128 via an SBUF-SBUF shift DMA.

Staging keeps the PE dense and overlaps the ACT-bound exp stream:
  stage 1: scores+exp+mask pair A (heads 0,1) with V/K projections and
           PV-A chn0 interleaved on the PE.
  stage 2: PV-A chn1 + scores pair B + PV-B chn0 per m-tile.
  stage 3: PV-B chn1 + output projection, with per-chunk normalization
           overlapped; ys evictions and y writeback ride the idle ACT
           engine/ring after the exp stream has drained.
"""

import numpy as np
import ml_dtypes

import concourse.bass as bass
import concourse.bacc as bacc
import concourse.mybir as mybir
import concourse.tile as tile
from concourse.bass_utils import run_bass_kernel_spmd

DIM = 1024
HEAD_DIM = 64
NUM_HEADS = 16
SCALE = HEAD_DIM**-0.5
B, N, M = 2, 1024, 2048
HPC = 4  # heads per core
E = HPC * HEAD_DIM  # 256: per-core projection width
P = 128
F32 = mybir.dt.float32
F16 = mybir.dt.float16
F8 = mybir.dt.float8e4
CT = DIM // P  # 8 contraction tiles
MT = M // P  # 16 m tiles
DR = mybir.MatmulPerfMode.DoubleRow


def build_program():
    nc = bacc.Bacc("TRN2", target_bir_lowering=False, debug=False, num_devices=8)

    # all activation/weight shards arrive contraction-major (pre-transposed)
    xT_d = nc.dram_tensor("xT", [DIM, N], F16, kind="ExternalInput").ap()
    ctxT_d = nc.dram_tensor("ctxT", [DIM, M], F16, kind="ExternalInput").ap()
    maskt_d = nc.dram_tensor("maskt", [M, N], F16, kind="ExternalInput").ap()
    # wq/wk columns DR-permuted on host: col t*128 + h*32 + i <- head h, d=t*32+i
    wqT_d = nc.dram_tensor("wqT", [DIM, E], F16, kind="ExternalInput").ap()
    wkT_d = nc.dram_tensor("wkT", [DIM, E], F16, kind="ExternalInput").ap()
    wvT_d = nc.dram_tensor("wvT", [DIM, E], F16, kind="ExternalInput").ap()
    woT_d = nc.dram_tensor("woT", [E, DIM], F16, kind="ExternalInput").ap()
    bk_d = nc.dram_tensor("bk", [E], F32, kind="ExternalInput").ap()
    y_d = nc.dram_tensor("y", [N, DIM], F16, kind="ExternalOutput").ap()

    Exp = mybir.ActivationFunctionType.Exp

    from contextlib import ExitStack

    with tile.TileContext(nc) as tc, ExitStack() as ctx:
        const = ctx.enter_context(tc.tile_pool(name="const", bufs=1))
        bk_sb = const.tile([P, E // P], F32)
        nc.sync.dma_start(out=bk_sb, in_=bk_d.rearrange("(t p) -> p t", p=P))

        persist = ctx.enter_context(tc.tile_pool(name="persist", bufs=1))
        qT8 = persist.tile([P, 2, 2, N], F8)  # [2hp x 64d, hl, t, n]
        kT8 = persist.tile([P, 2, M], F8)  # [2hp x 64d, t, m]
        vaug = persist.tile([P, MT, HPC, HEAD_DIM + 1], F16)
        woT = persist.tile([P, E // P, DIM], F16)
        otn2 = persist.tile([P, E // P, N], F16)

        # ones column: fill everything; v evictions overwrite cols 0:64
        nc.vector.memset(vaug, 1.0)
        # zero the partner-head q slots (cross-term killers for DR pairs)
        nc.gpsimd.memset(qT8[:, 0, 1, :], 0.0)
        nc.gpsimd.memset(qT8[:, 1, 0, :], 0.0)

        bwork = ctx.enter_context(tc.tile_pool(name="bwork", bufs=4))
        maskp = ctx.enter_context(tc.tile_pool(name="maskp", bufs=3))
        rbp = ctx.enter_context(tc.tile_pool(name="rbp", bufs=2))
        dnp = ctx.enter_context(tc.tile_pool(name="dnp", bufs=2))

        def emit_scores(spool, sbufs, h, mt, exmst, mk):
            """fp8 DoubleRow scores -> exp -> mask for head h at m-tile mt.
            One [128, 2, 512] psum tile per (h, mt); exp over the full 1024
            free in one ACT instr; one flat DVE mask-mul (2x_1p mode)."""
            hp, hl = divmod(h, 2)
            hs = slice(hp * 64, (hp + 1) * 64)
            st = spool.tile([P, 2, 512], F32, tag="st", name="st", bufs=sbufs)
            for chn in range(2):
                nc.tensor.matmul(
                    st[:, chn, :],
                    lhsT=kT8[hs, :, mt * P : (mt + 1) * P],
                    rhs=qT8[hs, hl, :, chn * 512 : (chn + 1) * 512],
                    start=True,
                    stop=True,
                    perf_mode=DR,
                )
            ex = bwork.tile([P, 2, 512], F16, tag="ex", name="ex")
            nc.scalar.activation(ex, st, Exp, scale=float(SCALE))
            nc.vector.tensor_mul(exmst[:, mt, hl, :], ex, mk)

        def emit_pv(ot_ps, hp, mt, exmst, chn):
            for hl in range(2):
                h = hp * 2 + hl
                nc.tensor.matmul(
                    ot_ps[hl],
                    lhsT=vaug[:, mt, h, :],
                    rhs=exmst[:, mt, hl, chn * 512 : (chn + 1) * 512],
                    start=(mt == 0),
                    stop=(mt == MT - 1),
                )

        def normalize_pair(hp, chn, ot_ps):
            """softmax-normalize head pair hp's n-half chn straight from the
            PV psum accumulators ot_ps (list: hl -> [65, 512] psum tile).
            Denominators (psum row 64) -> sbuf park -> partition 0 via shift
            DMA -> reciprocal -> partition_broadcast; one fused mul-evict per
            head writes normalized fp16 into otn2. Odd head shifts to
            partitions 64:128 via SBUF-SBUF DMA."""
            cs = slice(chn * 512, (chn + 1) * 512)
            dpk = dnp.tile([P, 2, 512], F32, tag="dpk", name="dpk")
            for hl in range(2):
                nc.vector.tensor_copy(dpk[64:65, hl, :], ot_ps[hl][64:65, :])
            dna = dnp.tile([1, 2, 512], F32, tag="dna", name="dna")
            nc.sync.dma_start(out=dna, in_=dpk[64:65, :, :])
            rca = rbp.tile([1, 2, 512], F32, tag="rca", name="rca")
            nc.vector.reciprocal_approx_fast(out=rca, in_=dna)
            rba = rbp.tile([HEAD_DIM, 2, 512], F32, tag="rba", name="rba")
            nc.gpsimd.partition_broadcast(rba, rca)
            nc.vector.tensor_mul(
                otn2[:HEAD_DIM, hp, cs], ot_ps[0][:HEAD_DIM, :], rba[:, 0, :]
            )
            tmp = rbp.tile([HEAD_DIM, 512], F16, tag="tmp", name="tmp")
            nc.vector.tensor_mul(tmp, ot_ps[1][:HEAD_DIM, :], rba[:, 1, :])
            # partition shift 0:64 -> 64:128 via SBUF-SBUF DMA
            nc.gpsimd.dma_start(out=otn2[HEAD_DIM:P, hp, cs], in_=tmp)

        def load_mask(mt, ring):
            mk = maskp.tile([P, N], F16, tag="mk", name="mk")
            ring.dma_start(out=mk, in_=maskt_d[mt * P : (mt + 1) * P, :])
            return mk

        with tc.tile_pool(name="exmp", bufs=1) as exmp:
            # masked exp(scores) parked per m-tile; one buffer reused across
            # head pairs (WAR: stage-2 rewrites a tile only after its PV read)
            exmst = exmp.tile([P, MT, 2, N], F16)

            with tc.tile_pool(name="wctx", bufs=1) as wctx_pool:
                wkT = wctx_pool.tile([P, CT, E], F16)
                wvT = wctx_pool.tile([P, CT, E], F16)
                ctxT = wctx_pool.tile([P, CT, M], F16)

                with tc.tile_pool(name="qx", bufs=1) as qx_pool:
                    wqT = qx_pool.tile([P, CT, E], F16)
                    xT = qx_pool.tile([P, CT, N], F16)
                    # DMA rings are issue-rate bound (~0.6us/instr): few,
                    # big instructions, dependency-first. The scalar (ACT)
                    # ring is idle until the first exp, so the bulk input
                    # loads go there. ctx is issued m-quarter-major so K
                    # chunk 0 unblocks early.
                    nc.sync.dma_start(
                        out=wqT, in_=wqT_d.rearrange("(c p) e -> p c e", p=P)
                    )
                    for jp in range(CT // 2):
                        nc.sync.dma_start(
                            out=xT[:, 2 * jp : 2 * jp + 2, :],
                            in_=xT_d[jp * 2 * P : (jp + 1) * 2 * P, :].rearrange(
                                "(c p) n -> p c n", p=P
                            ),
                        )
                    nc.scalar.dma_start(
                        out=wkT, in_=wkT_d.rearrange("(c p) e -> p c e", p=P)
                    )
                    for q in range(4):
                        for jp in range(CT // 2):
                            nc.scalar.dma_start(
                                out=ctxT[
                                    :, 2 * jp : 2 * jp + 2, q * 512 : (q + 1) * 512
                                ],
                                in_=ctxT_d[
                                    jp * 2 * P : (jp + 1) * 2 * P,
                                    q * 512 : (q + 1) * 512,
                                ].rearrange("(c p) m -> p c m", p=P),
                            )
                    nc.scalar.dma_start(
                        out=wvT, in_=wvT_d.rearrange("(c p) e -> p c e", p=P)
                    )
                    nc.scalar.dma_start(
                        out=woT, in_=woT_d.rearrange("(c p) e -> p c e", p=P)
                    )

                    # Q projection, contraction-chunk outer: the PE consumes
                    # x chunks as they land; evictions cast psum -> fp8 DR
                    # layout (partition-aligned thanks to the host-side wq
                    # column permutation).
                    qgroups = [
                        (et, chn) for et in range(E // P) for chn in range(N // 512)
                    ]
                    with tc.tile_pool(name="ppsA", bufs=1, space="PSUM") as ppsA:
                        pqs = {
                            g: ppsA.tile([P, 512], F32, tag=f"pq{i}", name=f"pq{i}")
                            for i, g in enumerate(qgroups)
                        }
                        for j in range(CT):
                            for et, chn in qgroups:
                                nc.tensor.matmul(
                                    pqs[(et, chn)],
                                    lhsT=wqT[:, j, et * P : (et + 1) * P],
                                    rhs=xT[:, j, chn * 512 : (chn + 1) * 512],
                                    start=(j == 0),
                                    stop=(j == CT - 1),
                                )
                        for et, chn in qgroups:
                            # psum block et holds the parity-et heads; land
                            # them in their own t-slot (t == hl == et)
                            nc.vector.tensor_copy(
                                qT8[:, et, et, chn * 512 : (chn + 1) * 512],
                                pqs[(et, chn)],
                            )

                def emit_kproj(kps, et, chm):
                    pk = kps.tile([P, 512], F32, tag="pk", name="pk")
                    for j in range(CT):
                        nc.tensor.matmul(
                            pk,
                            lhsT=wkT[:, j, et * P : (et + 1) * P],
                            rhs=ctxT[:, j, chm * 512 : (chm + 1) * 512],
                            start=(j == 0),
                            stop=(j == CT - 1),
                        )
                    nc.vector.tensor_scalar_add(
                        kT8[:, et, chm * 512 : (chm + 1) * 512],
                        pk,
                        bk_sb[:, et : et + 1],
                    )

                # stage 1: scores+exp+mask pair A (ACT-bound) with the V and
                # K projections and PV-A chn0 interleaved on the PE. Both et
                # chunks of a K m-quarter must land before its scores (DR
                # reads t=0 and t=1 together). PSUM: sps1 4 + vps 1 + kps 1
                # + opsA0 2 = 8 banks.
                with (
                    tc.tile_pool(name="sps1", bufs=1, space="PSUM") as sps1,
                    tc.tile_pool(name="vps", bufs=1, space="PSUM") as vps,
                    tc.tile_pool(name="kps", bufs=1, space="PSUM") as kps,
                    tc.tile_pool(name="opsA0", bufs=1, space="PSUM") as opsA0,
                ):
                    ot_psA0 = [
                        opsA0.tile(
                            [HEAD_DIM + 1, 512], F32, tag=f"a0{i}", name=f"a0{i}"
                        )
                        for i in range(2)
                    ]
                    emit_kproj(kps, 0, 0)
                    emit_kproj(kps, 1, 0)
                    for mt in range(MT):
                        if mt % 4 == 2:
                            chm = mt // 4 + 1
                            if chm < 4:
                                emit_kproj(kps, 0, chm)
                                emit_kproj(kps, 1, chm)
                        mk = load_mask(mt, nc.sync)
                        for hl in range(2):
                            emit_scores(sps1, 2, hl, mt, exmst, mk)
                        pv = vps.tile([P, HPC, HEAD_DIM], F32, tag="pv")
                        for j in range(CT):
                            nc.tensor.matmul(
                                pv,
                                lhsT=ctxT[:, j, mt * P : (mt + 1) * P],
                                rhs=wvT[:, j, :],
                                start=(j == 0),
                                stop=(j == CT - 1),
                            )
                        # batched strided v eviction (Pool can't read PSUM)
                        nc.vector.tensor_copy(vaug[:, mt, :, :HEAD_DIM], pv)
                        emit_pv(ot_psA0, 0, mt, exmst, 0)
                    # normalize A chn0 straight from psum; overlaps stage-2
                    # PE work (stage-2 allocations wait only on these banks)
                    normalize_pair(0, 0, ot_psA0)

            # stage 2: PV-A chn1 + scores pair B + PV-B chn0 per m-tile.
            # PSUM: sps2 4 + opsA1 2 + opsB0 2 = 8 banks (opsA0 drains into
            # the first iterations).
            with (
                tc.tile_pool(name="sps2", bufs=1, space="PSUM") as sps2,
                tc.tile_pool(name="opsA1", bufs=1, space="PSUM") as opsA1,
                tc.tile_pool(name="opsB0", bufs=1, space="PSUM") as opsB0,
            ):
                ot_psA1 = [
                    opsA1.tile(
                        [HEAD_DIM + 1, 512], F32, tag=f"a1{i}", name=f"a1{i}"
                    )
                    for i in range(2)
                ]
                ot_psB0 = [
                    opsB0.tile(
                        [HEAD_DIM + 1, 512], F32, tag=f"b0{i}", name=f"b0{i}"
                    )
                    for i in range(2)
                ]
                for mt in range(MT):
                    mk = load_mask(mt, nc.gpsimd)
                    emit_pv(ot_psA1, 0, mt, exmst, 1)
                    for hl in range(2):
                        emit_scores(sps2, 2, 2 + hl, mt, exmst, mk)
                    emit_pv(ot_psB0, 1, mt, exmst, 0)
                normalize_pair(0, 1, ot_psA1)
                normalize_pair(1, 0, ot_psB0)

            # stage 3: PV-B chn1 interleaved with the first half of the
            # output projection; ys evictions + y DMA ride the now-idle ACT
            # engine/ring.
            with (
                tc.tile_pool(name="opsB1", bufs=1, space="PSUM") as opsB1,
                tc.tile_pool(name="ypsum", bufs=3, space="PSUM") as ypsum,
                tc.tile_pool(name="ypool", bufs=3) as ypool,
            ):
                def emit_oproj(nb):
                    ys = ypool.tile([P, DIM], F16, tag="ys", name="ys")
                    for oc in range(DIM // 512):
                        yp = ypsum.tile([P, 512], F32, tag="yp", name="yp")
                        for hp in range(E // P):
                            nc.tensor.matmul(
                                yp,
                                lhsT=otn2[:, hp, nb * P : (nb + 1) * P],
                                rhs=woT[:, hp, oc * 512 : (oc + 1) * 512],
                                start=(hp == 0),
                                stop=(hp == E // P - 1),
                            )
                        nc.scalar.copy(ys[:, oc * 512 : (oc + 1) * 512], yp)
                    nc.scalar.dma_start(out=y_d[nb * P : (nb + 1) * P, :], in_=ys)

                ot_psB1 = [
                    opsB1.tile(
                        [HEAD_DIM + 1, 512], F32, tag=f"b1{i}", name=f"b1{i}"
                    )
                    for i in range(2)
                ]
                for mt in range(MT):
                    emit_pv(ot_psB1, 1, mt, exmst, 1)
                    if mt >= 9 and mt % 2 == 1:
                        emit_oproj((mt - 9) // 2)
                emit_oproj(3)
                normalize_pair(1, 1, ot_psB1)
                for nb in range(4, N // P):
                    emit_oproj(nb)

    nc.compile()
    return nc


_NC_CACHE = []


def _get_nc():
    if not _NC_CACHE:
        _NC_CACHE.append(build_program())
    return _NC_CACHE[0]


# DR column permutation: psum block a holds the parity-a heads,
# new col a*128 + hp*64 + d <- orig col (2*hp + a)*64 + d
_DR_PERM = np.array(
    [(2 * hp + a) * 64 + d for a in range(2) for hp in range(2) for d in range(64)]
)


def make_in_maps(x, context, mask, Wq, Wkv, b_kv, Wo):
    f16 = np.float16
    x = np.asarray(x, dtype=np.float32)
    context = np.asarray(context, dtype=np.float32)
    mask = np.asarray(mask)
    Wq = np.asarray(Wq, dtype=np.float32)
    Wkv = np.asarray(Wkv, dtype=np.float32)
    b_kv = np.asarray(b_kv, dtype=np.float32)
    Wo = np.asarray(Wo, dtype=np.float32)

    in_maps = []
    for b in range(B):
        xtb = np.ascontiguousarray(x[b].T).astype(f16)
        ctb = np.ascontiguousarray(context[b].T).astype(f16)
        mtb = np.ascontiguousarray(mask[b].T).astype(f16)
        for g in range(NUM_HEADS // HPC):
            sl = slice(E * g, E * (g + 1))
            wq_g = Wq[sl][_DR_PERM]
            wk_g = Wkv[sl][_DR_PERM]
            bk_g = b_kv[sl][_DR_PERM]
            in_maps.append(
                {
                    "xT": xtb,
                    "ctxT": ctb,
                    "maskt": mtb,
                    "wqT": np.ascontiguousarray(wq_g.T).astype(f16),
                    "wkT": np.ascontiguousarray(wk_g.T).astype(f16),
                    "wvT": np.ascontiguousarray(
                        Wkv[DIM + E * g : DIM + E * (g + 1)].T
                    ).astype(f16),
                    "woT": np.ascontiguousarray(Wo[:, sl].T).astype(f16),
                    "bk": np.ascontiguousarray(bk_g),
                }
            )
    return in_maps


def combine_outputs(ys, b_kv, Wo):
    """ys: list of 8 per-core partial outputs [N, DIM], core order (b, g)."""
    b_v = np.asarray(b_kv, dtype=np.float32)[DIM:]
    ybias = np.asarray(Wo, dtype=np.float32) @ b_v  # [DIM]
    out = np.empty((B, N, DIM), dtype=np.float32)
    G = NUM_HEADS // HPC
    for b in range(B):
        acc = np.asarray(ys[G * b], dtype=np.float32)
        for g in range(1, G):
            acc = acc + np.asarray(ys[G * b + g], dtype=np.float32)
        out[b] = acc + ybias[None, :]
    return out


def kernel(x, context, mask, Wq, Wkv, b_kv, Wo):
    nc = _get_nc()
    in_maps = make_in_maps(x, context, mask, Wq, Wkv, b_kv, Wo)
    res = run_bass_kernel_spmd(nc, in_maps, core_ids=list(range(8)))
    ys = [m["y"] for m in res.results]
    return combine_outputs(ys, b_kv, Wo)


# revision 11
# speedup vs baseline: 1.0397x; 1.0397x over previous
"""CrossAttention Trainium2 kernel (8 NeuronCores, SPMD).

Sharding: data-parallel over batch B=2, tensor-parallel over the 16 heads in
4 groups of 4 heads -> 8 cores, one (batch, head-group) pair each. Each core
computes its 4 heads' Q/K/V projections, masked softmax cross-attention, and
its partial output projection y_g = softmax(q k^T * scale) v @ Wo[:, g].T.
The host sums the 4 partial outputs per batch (the Wo row-split all-reduce,
done at unshard time) and adds the v-bias term Wo @ b_v, which is constant
across rows and factors out of the attention (softmax rows sum to 1).

Numerics: fp16 matmuls with fp32 PSUM accumulation (fp16 costs the same as
bf16 on every engine and carries 3 extra mantissa bits; all tensors here are
comfortably inside fp16 range). x travels as fp8e4m3, halving its DMA
footprint; the quantization only perturbs q and thus the softmax weights,
damped by the small score scale. The PE is row-stream bound (cycles = moving
free size regardless of contraction width or dtype), so fp8 DoubleRow
matmuls are NOT used: measured on hardware they process 2x the moving rows
for the same output, a net loss.

Layout: activations and weights arrive contraction-major (pre-transposed on
the host) so every device DMA is a contiguous row load. Scores are computed
transposed: ST[m, n] per head, so the PV matmul contracts over m directly,
and an appended ones-column on the V stationary operand yields the softmax
denominator for free. exp() is unnormalized (no max subtraction; scores are
bounded); mask zeros are applied multiplicatively after exp (DVE, 2x mode).

Normalization happens straight out of the PV PSUM accumulators (no fp32
park): denominator row 64 -> partition 0 via a tiny SBUF shift DMA,
reciprocal (DVE), partition_broadcast (Pool), then one fused multiply-evict
per head writes normalized fp16 into otn2. Odd heads reach partitions 64:128
via an SBUF-SBUF shift DMA.

Schedule: the kernel is PE-bound (~230K PE cycles vs ~72us of exp on ACT),
so PV work is pulled forward under the exp stream instead of trailing it:
  stage 1: scores+exp+mask pair A with the V and K projections and PV-A
           chn0 interleaved on the PE.
  stage 2: PV-A chn1 + scores pair B + PV-B chn0 per m-tile.
  stage 3: PV-B chn1 overlapped with the first half of the output
           projection; ys evictions + y DMA ride the ACT engine/ring, idle
           once the exp stream has drained.
Input DMAs are spread across all three rings (sync/scalar/gpsimd) with
x j-chunked so the Q projection starts as early as possible.
"""

import numpy as np
import ml_dtypes

import concourse.bass as bass
import concourse.bacc as bacc
import concourse.mybir as mybir
import concourse.tile as tile
from concourse.bass_utils import run_bass_kernel_spmd

DIM = 1024
HEAD_DIM = 64
NUM_HEADS = 16
SCALE = HEAD_DIM**-0.5
B, N, M = 2, 1024, 2048
HPC = 4  # heads per core
E = HPC * HEAD_DIM  # 256: per-core projection width
P = 128
F32 = mybir.dt.float32
F16 = mybir.dt.float16
F8 = mybir.dt.float8e4
CT = DIM // P  # 8 contraction tiles
MT = M // P  # 16 m tiles


def build_program():
    nc = bacc.Bacc("TRN2", target_bir_lowering=False, debug=False, num_devices=8)

    # all activation/weight shards arrive contraction-major (pre-transposed)
    xT_d = nc.dram_tensor("xT", [DIM, N], F8, kind="ExternalInput").ap()
    ctxT_d = nc.dram_tensor("ctxT", [DIM, M], F16, kind="ExternalInput").ap()
    maskt_d = nc.dram_tensor("maskt", [M, N], F16, kind="ExternalInput").ap()
    wqT_d = nc.dram_tensor("wqT", [DIM, E], F16, kind="ExternalInput").ap()
    wkT_d = nc.dram_tensor("wkT", [DIM, E], F16, kind="ExternalInput").ap()
    wvT_d = nc.dram_tensor("wvT", [DIM, E], F16, kind="ExternalInput").ap()
    woT_d = nc.dram_tensor("woT", [E, DIM], F16, kind="ExternalInput").ap()
    bk_d = nc.dram_tensor("bk", [E], F32, kind="ExternalInput").ap()
    y_d = nc.dram_tensor("y", [N, DIM], F16, kind="ExternalOutput").ap()

    Exp = mybir.ActivationFunctionType.Exp

    from contextlib import ExitStack

    with tile.TileContext(nc) as tc, ExitStack() as ctx:
        const = ctx.enter_context(tc.tile_pool(name="const", bufs=1))
        bk_sb = const.tile([P, E // P], F32)
        nc.sync.dma_start(out=bk_sb, in_=bk_d.rearrange("(t p) -> p t", p=P))

        persist = ctx.enter_context(tc.tile_pool(name="persist", bufs=1))
        qT = persist.tile([P, E // P, N], F16)
        kT = persist.tile([P, E // P, M], F16)
        vaug = persist.tile([P, MT, HPC, HEAD_DIM + 1], F16)
        woT = persist.tile([P, E // P, DIM], F16)
        otn2 = persist.tile([P, E // P, N], F16)

        # ones column: fill everything; v evictions overwrite cols 0:64
        nc.vector.memset(vaug, 1.0)

        bwork = ctx.enter_context(tc.tile_pool(name="bwork", bufs=4))
        maskp = ctx.enter_context(tc.tile_pool(name="maskp", bufs=3))
        rbp = ctx.enter_context(tc.tile_pool(name="rbp", bufs=2))
        dnp = ctx.enter_context(tc.tile_pool(name="dnp", bufs=2))

        def emit_scores(spool, sbufs, h, mt, exmst, mk):
            """scores -> exp -> mask for head h at m-tile mt. One
            [128, 2, 512] psum tile per (h, mt); exp over the full 1024 free
            in one ACT instr; one flat DVE mask-mul (2x_1p mode)."""
            hp, hl = divmod(h, 2)
            erow = slice(hl * HEAD_DIM, (hl + 1) * HEAD_DIM)
            st = spool.tile([P, 2, 512], F32, tag="st", name="st", bufs=sbufs)
            for chn in range(2):
                nc.tensor.matmul(
                    st[:, chn, :],
                    lhsT=kT[erow, hp, mt * P : (mt + 1) * P],
                    rhs=qT[erow, hp, chn * 512 : (chn + 1) * 512],
                    start=True,
                    stop=True,
                )
            ex = bwork.tile([P, 2, 512], F16, tag="ex", name="ex")
            nc.scalar.activation(ex, st, Exp, scale=float(SCALE))
            nc.vector.tensor_mul(exmst[:, mt, hl, :], ex, mk)

        def emit_pv(ot_ps, hp, mt, exmst, chn):
            for hl in range(2):
                h = hp * 2 + hl
                nc.tensor.matmul(
                    ot_ps[hl],
                    lhsT=vaug[:, mt, h, :],
                    rhs=exmst[:, mt, hl, chn * 512 : (chn + 1) * 512],
                    start=(mt == 0),
                    stop=(mt == MT - 1),
                )

        def normalize_pair(hp, chn, ot_ps):
            """softmax-normalize head pair hp's n-half chn straight from the
            PV psum accumulators ot_ps (list: hl -> [65, 512] psum tile).
            Denominators (psum row 64) -> sbuf park -> partition 0 via shift
            DMA -> reciprocal -> partition_broadcast; one fused mul-evict per
            head writes normalized fp16 into otn2. Odd head shifts to
            partitions 64:128 via SBUF-SBUF DMA."""
            cs = slice(chn * 512, (chn + 1) * 512)
            dpk = dnp.tile([P, 2, 512], F32, tag="dpk", name="dpk")
            for hl in range(2):
                nc.vector.tensor_copy(dpk[64:65, hl, :], ot_ps[hl][64:65, :])
            dna = dnp.tile([1, 2, 512], F32, tag="dna", name="dna")
            nc.sync.dma_start(out=dna, in_=dpk[64:65, :, :])
            rca = rbp.tile([1, 2, 512], F32, tag="rca", name="rca")
            nc.vector.reciprocal_approx_fast(out=rca, in_=dna)
            rba = rbp.tile([HEAD_DIM, 2, 512], F32, tag="rba", name="rba")
            nc.gpsimd.partition_broadcast(rba, rca)
            nc.vector.tensor_mul(
                otn2[:HEAD_DIM, hp, cs], ot_ps[0][:HEAD_DIM, :], rba[:, 0, :]
            )
            tmp = rbp.tile([HEAD_DIM, 512], F16, tag="tmp", name="tmp")
            nc.vector.tensor_mul(tmp, ot_ps[1][:HEAD_DIM, :], rba[:, 1, :])
            # partition shift 0:64 -> 64:128 via SBUF-SBUF DMA
            nc.gpsimd.dma_start(out=otn2[HEAD_DIM:P, hp, cs], in_=tmp)

        def load_mask(mt, ring):
            mk = maskp.tile([P, N], F16, tag="mk", name="mk")
            ring.dma_start(out=mk, in_=maskt_d[mt * P : (mt + 1) * P, :])
            return mk

        with tc.tile_pool(name="exmp", bufs=1) as exmp:
            # masked exp(scores) parked per m-tile; one buffer reused across
            # head pairs (WAR: stage-2 rewrites a tile only after its PV read)
            exmst = exmp.tile([P, MT, 2, N], F16)

            with tc.tile_pool(name="wctx", bufs=1) as wctx_pool:
                wkT = wctx_pool.tile([P, CT, E], F16)
                wvT = wctx_pool.tile([P, CT, E], F16)
                ctxT = wctx_pool.tile([P, CT, M], F16)

                with tc.tile_pool(name="qx", bufs=1) as qx_pool:
                    wqT = qx_pool.tile([P, CT, E], F16)
                    xT = qx_pool.tile([P, CT, N], F8)
                    # DMA rings are issue-rate bound (~0.6us/instr) and each
                    # sustains only ~140 GB/s, so the prologue spreads the
                    # critical loads across all three rings, dependency-first:
                    #   sync:   wq, then x j-chunks (Q proj gate)
                    #   scalar: wk, ctx m-quarters (K proj gate), wv, wo
                    #   gpsimd: x tail chunks
                    nc.sync.dma_start(
                        out=wqT, in_=wqT_d.rearrange("(c p) e -> p c e", p=P)
                    )
                    for jp in range(2):
                        nc.sync.dma_start(
                            out=xT[:, 2 * jp : 2 * jp + 2, :],
                            in_=xT_d[jp * 2 * P : (jp + 1) * 2 * P, :].rearrange(
                                "(c p) n -> p c n", p=P
                            ),
                        )
                    for jp in range(2, 4):
                        nc.gpsimd.dma_start(
                            out=xT[:, 2 * jp : 2 * jp + 2, :],
                            in_=xT_d[jp * 2 * P : (jp + 1) * 2 * P, :].rearrange(
                                "(c p) n -> p c n", p=P
                            ),
                        )
                    nc.scalar.dma_start(
                        out=wkT, in_=wkT_d.rearrange("(c p) e -> p c e", p=P)
                    )
                    for q in range(4):
                        for jp in range(CT // 2):
                            nc.scalar.dma_start(
                                out=ctxT[
                                    :, 2 * jp : 2 * jp + 2, q * 512 : (q + 1) * 512
                                ],
                                in_=ctxT_d[
                                    jp * 2 * P : (jp + 1) * 2 * P,
                                    q * 512 : (q + 1) * 512,
                                ].rearrange("(c p) m -> p c m", p=P),
                            )
                    nc.scalar.dma_start(
                        out=wvT, in_=wvT_d.rearrange("(c p) e -> p c e", p=P)
                    )
                    nc.scalar.dma_start(
                        out=woT, in_=woT_d.rearrange("(c p) e -> p c e", p=P)
                    )

                    # Q projection, contraction-chunk outer: the PE consumes
                    # x chunks as they land
                    qgroups = [
                        (et, chn) for et in range(E // P) for chn in range(N // 512)
                    ]
                    with tc.tile_pool(name="ppsA", bufs=1, space="PSUM") as ppsA:
                        pqs = {
                            g: ppsA.tile([P, 512], F32, tag=f"pq{i}", name=f"pq{i}")
                            for i, g in enumerate(qgroups)
                        }
                        for j in range(CT):
                            for et, chn in qgroups:
                                nc.tensor.matmul(
                                    pqs[(et, chn)],
                                    lhsT=wqT[:, j, et * P : (et + 1) * P],
                                    rhs=xT[:, j, chn * 512 : (chn + 1) * 512],
                                    start=(j == 0),
                                    stop=(j == CT - 1),
                                )
                        for et, chn in qgroups:
                            nc.vector.tensor_copy(
                                qT[:, et, chn * 512 : (chn + 1) * 512],
                                pqs[(et, chn)],
                            )

                def emit_kproj(kps, et, chm):
                    pk = kps.tile([P, 512], F32, tag="pk", name="pk")
                    for j in range(CT):
                        nc.tensor.matmul(
                            pk,
                            lhsT=wkT[:, j, et * P : (et + 1) * P],
                            rhs=ctxT[:, j, chm * 512 : (chm + 1) * 512],
                            start=(j == 0),
                            stop=(j == CT - 1),
                        )
                    nc.vector.tensor_scalar_add(
                        kT[:, et, chm * 512 : (chm + 1) * 512],
                        pk,
                        bk_sb[:, et : et + 1],
                    )

                # stage 1: scores+exp+mask pair A (ACT-bound) with the V and
                # K projections and PV-A chn0 interleaved on the PE. Pair A
                # needs only kT et0; et1 chunks are paced ahead of stage 2.
                # PSUM: sps1 4 + vps 1 + kps 1 + opsA0 2 = 8 banks.
                with (
                    tc.tile_pool(name="sps1", bufs=1, space="PSUM") as sps1,
                    tc.tile_pool(name="vps", bufs=1, space="PSUM") as vps,
                    tc.tile_pool(name="kps", bufs=1, space="PSUM") as kps,
                    tc.tile_pool(name="opsA0", bufs=1, space="PSUM") as opsA0,
                ):
                    ot_psA0 = [
                        opsA0.tile(
                            [HEAD_DIM + 1, 512], F32, tag=f"a0{i}", name=f"a0{i}"
                        )
                        for i in range(2)
                    ]
                    emit_kproj(kps, 0, 0)
                    for mt in range(MT):
                        # keep kT(et0) one chunk ahead of the scores that
                        # consume it; kT(et1) lands before stage 2
                        if mt % 2 == 0:
                            et, chm = divmod(mt // 2 + 1, M // 512)
                            if et < 2:
                                emit_kproj(kps, et, chm)
                        mk = load_mask(mt, nc.sync)
                        for hl in range(2):
                            emit_scores(sps1, 2, hl, mt, exmst, mk)
                        pv = vps.tile([P, HPC, HEAD_DIM], F32, tag="pv")
                        for j in range(CT):
                            nc.tensor.matmul(
                                pv,
                                lhsT=ctxT[:, j, mt * P : (mt + 1) * P],
                                rhs=wvT[:, j, :],
                                start=(j == 0),
                                stop=(j == CT - 1),
                            )
                        # batched strided v eviction (Pool can't read PSUM)
                        nc.vector.tensor_copy(vaug[:, mt, :, :HEAD_DIM], pv)
                        emit_pv(ot_psA0, 0, mt, exmst, 0)
                    # normalize A chn0 straight from psum; overlaps stage-2
                    # PE work (stage-2 allocations wait only on these banks)
                    normalize_pair(0, 0, ot_psA0)

            # stage 2: PV-A chn1 + scores pair B + PV-B chn0 per m-tile.
            # PSUM: sps2 4 + opsA1 2 + opsB0 2 = 8 banks (opsA0 drains into
            # the first iterations).
            with (
                tc.tile_pool(name="sps2", bufs=1, space="PSUM") as sps2,
                tc.tile_pool(name="opsA1", bufs=1, space="PSUM") as opsA1,
                tc.tile_pool(name="opsB0", bufs=1, space="PSUM") as opsB0,
            ):
                ot_psA1 = [
                    opsA1.tile(
                        [HEAD_DIM + 1, 512], F32, tag=f"a1{i}", name=f"a1{i}"
                    )
                    for i in range(2)
                ]
                ot_psB0 = [
                    opsB0.tile(
                        [HEAD_DIM + 1, 512], F32, tag=f"b0{i}", name=f"b0{i}"
                    )
                    for i in range(2)
                ]
                for mt in range(MT):
                    mk = load_mask(mt, nc.gpsimd)
                    emit_pv(ot_psA1, 0, mt, exmst, 1)
                    for hl in range(2):
                        emit_scores(sps2, 2, 2 + hl, mt, exmst, mk)
                    emit_pv(ot_psB0, 1, mt, exmst, 0)
                normalize_pair(0, 1, ot_psA1)
                normalize_pair(1, 0, ot_psB0)

            # stage 3: PV-B chn1 interleaved with the first half of the
            # output projection; ys evictions + y DMA ride the now-idle ACT
            # engine/ring.
            with (
                tc.tile_pool(name="opsB1", bufs=1, space="PSUM") as opsB1,
                tc.tile_pool(name="ypsum", bufs=3, space="PSUM") as ypsum,
                tc.tile_pool(name="ypool", bufs=3) as ypool,
            ):
                def emit_oproj(nb):
                    ys = ypool.tile([P, DIM], F16, tag="ys", name="ys")
                    for oc in range(DIM // 512):
                        yp = ypsum.tile([P, 512], F32, tag="yp", name="yp")
                        for hp in range(E // P):
                            nc.tensor.matmul(
                                yp,
                                lhsT=otn2[:, hp, nb * P : (nb + 1) * P],
                                rhs=woT[:, hp, oc * 512 : (oc + 1) * 512],
                                start=(hp == 0),
                                stop=(hp == E // P - 1),
                            )
                        nc.scalar.copy(ys[:, oc * 512 : (oc + 1) * 512], yp)
                    nc.scalar.dma_start(out=y_d[nb * P : (nb + 1) * P, :], in_=ys)

                ot_psB1 = [
                    opsB1.tile(
                        [HEAD_DIM + 1, 512], F32, tag=f"b1{i}", name=f"b1{i}"
                    )
                    for i in range(2)
                ]
                for mt in range(MT):
                    emit_pv(ot_psB1, 1, mt, exmst, 1)
                    if mt >= 9 and mt % 2 == 1:
                        emit_oproj((mt - 9) // 2)
                emit_oproj(3)
                normalize_pair(1, 1, ot_psB1)
                for nb in range(4, N // P):
                    emit_oproj(nb)

    nc.compile()
    return nc


_NC_CACHE = []


def _get_nc():
    if not _NC_CACHE:
        _NC_CACHE.append(build_program())
    return _NC_CACHE[0]


def make_in_maps(x, context, mask, Wq, Wkv, b_kv, Wo):
    f16 = np.float16
    f8 = ml_dtypes.float8_e4m3
    x = np.asarray(x, dtype=np.float32)
    context = np.asarray(context, dtype=np.float32)
    mask = np.asarray(mask)
    Wq = np.asarray(Wq, dtype=np.float32)
    Wkv = np.asarray(Wkv, dtype=np.float32)
    b_kv = np.asarray(b_kv, dtype=np.float32)
    Wo = np.asarray(Wo, dtype=np.float32)

    in_maps = []
    for b in range(B):
        xtb = np.ascontiguousarray(x[b].T).astype(f8)
        ctb = np.ascontiguousarray(context[b].T).astype(f16)
        mtb = np.ascontiguousarray(mask[b].T).astype(f16)
        for g in range(NUM_HEADS // HPC):
            sl = slice(E * g, E * (g + 1))
            in_maps.append(
                {
                    "xT": xtb,
                    "ctxT": ctb,
                    "maskt": mtb,
                    "wqT": np.ascontiguousarray(Wq[sl].T).astype(f16),
                    "wkT": np.ascontiguousarray(Wkv[sl].T).astype(f16),
                    "wvT": np.ascontiguousarray(
                        Wkv[DIM + E * g : DIM + E * (g + 1)].T
                    ).astype(f16),
                    "woT": np.ascontiguousarray(Wo[:, sl].T).astype(f16),
                    "bk": np.ascontiguousarray(b_kv[sl]),
                }
            )
    return in_maps


def combine_outputs(ys, b_kv, Wo):
    """ys: list of 8 per-core partial outputs [N, DIM], core order (b, g)."""
    b_v = np.asarray(b_kv, dtype=np.float32)[DIM:]
    ybias = np.asarray(Wo, dtype=np.float32) @ b_v  # [DIM]
    out = np.empty((B, N, DIM), dtype=np.float32)
    G = NUM_HEADS // HPC
    for b in range(B):
        acc = np.asarray(ys[G * b], dtype=np.float32)
        for g in range(1, G):
            acc = acc + np.asarray(ys[G * b + g], dtype=np.float32)
        out[b] = acc + ybias[None, :]
    return out


def kernel(x, context, mask, Wq, Wkv, b_kv, Wo):
    nc = _get_nc()
    in_maps = make_in_maps(x, context, mask, Wq, Wkv, b_kv, Wo)
    res = run_bass_kernel_spmd(nc, in_maps, core_ids=list(range(8)))
    ys = [m["y"] for m in res.results]
    return combine_outputs(ys, b_kv, Wo)


# revision 12
# speedup vs baseline: 1.1973x; 1.1516x over previous
"""CrossAttention Trainium2 kernel (8 NeuronCores, SPMD).

Sharding: data-parallel over batch B=2, tensor-parallel over the 16 heads in
4 groups of 4 heads -> 8 cores, one (batch, head-group) pair each. Each core
computes its 4 heads' Q/K/V projections, masked softmax cross-attention, and
its partial output projection y_g = softmax(q k^T * scale) v @ Wo[:, g].T.
The host sums the 4 partial outputs per batch (the Wo row-split all-reduce,
done at unshard time) and adds the v-bias term Wo @ b_v, which is constant
across rows and factors out of the attention (softmax rows sum to 1).

Numerics: fp16 matmuls with fp32 PSUM accumulation (fp16 costs the same as
bf16 on every engine and carries 3 extra mantissa bits; all tensors here are
comfortably inside fp16 range). x travels as fp8e4m3, halving its DMA
footprint; the quantization only perturbs q and thus the softmax weights,
damped by the small score scale. The PE is row-stream bound (cycles = moving
free size regardless of contraction width or dtype), so fp8 DoubleRow
matmuls are NOT used: measured on hardware they process 2x the moving rows
for the same output, a net loss.

Layout: activations and weights arrive contraction-major (pre-transposed on
the host) so every device DMA is a contiguous row load. Scores are computed
transposed: ST[m, n] per head, so the PV matmul contracts over m directly,
and an appended ones-column on the V stationary operand yields the softmax
denominator for free. exp() is unnormalized (no max subtraction; scores are
bounded); mask zeros are applied multiplicatively after exp (DVE, 2x mode).

Normalization happens straight out of the PV PSUM accumulators (no fp32
park): denominator row 64 -> partition 0 via a tiny SBUF shift DMA,
reciprocal (DVE), partition_broadcast (Pool), then one fused multiply-evict
per head writes normalized fp16 into otn2. Odd heads reach partitions 64:128
via an SBUF-SBUF shift DMA.

Schedule: the kernel is PE-bound (~230K PE cycles vs ~72us of exp on ACT),
so PV work is pulled forward under the exp stream instead of trailing it:
  stage 1: scores+exp+mask pair A with the V and K projections and PV-A
           chn0 interleaved on the PE.
  stage 2: PV-A chn1 + scores pair B + PV-B chn0 per m-tile.
  stage 3: PV-B chn1 overlapped with the first half of the output
           projection; ys evictions + y DMA ride the ACT engine/ring, idle
           once the exp stream has drained.
Input DMAs are spread across all three rings (sync/scalar/gpsimd) with
x j-chunked so the Q projection starts as early as possible.
"""

import numpy as np
import ml_dtypes

import concourse.bass as bass
import concourse.bacc as bacc
import concourse.mybir as mybir
import concourse.tile as tile
from concourse.bass_utils import run_bass_kernel_spmd

DIM = 1024
HEAD_DIM = 64
NUM_HEADS = 16
SCALE = HEAD_DIM**-0.5
B, N, M = 2, 1024, 2048
HPC = 4  # heads per core
E = HPC * HEAD_DIM  # 256: per-core projection width
P = 128
F32 = mybir.dt.float32
F16 = mybir.dt.bfloat16  # bf16: measured ~20% faster per matmul than fp16 on HW
F8 = mybir.dt.float8e4
CT = DIM // P  # 8 contraction tiles
MT = M // P  # 16 m tiles


def build_program():
    nc = bacc.Bacc("TRN2", target_bir_lowering=False, debug=False, num_devices=8)

    # all activation/weight shards arrive contraction-major (pre-transposed)
    xT_d = nc.dram_tensor("xT", [DIM, N], F8, kind="ExternalInput").ap()
    ctxT_d = nc.dram_tensor("ctxT", [DIM, M], F16, kind="ExternalInput").ap()
    maskt_d = nc.dram_tensor("maskt", [M, N], F16, kind="ExternalInput").ap()
    wqT_d = nc.dram_tensor("wqT", [DIM, E], F16, kind="ExternalInput").ap()
    wkT_d = nc.dram_tensor("wkT", [DIM, E], F16, kind="ExternalInput").ap()
    wvT_d = nc.dram_tensor("wvT", [DIM, E], F16, kind="ExternalInput").ap()
    woT_d = nc.dram_tensor("woT", [E, DIM], F16, kind="ExternalInput").ap()
    bk_d = nc.dram_tensor("bk", [E], F32, kind="ExternalInput").ap()
    y_d = nc.dram_tensor("y", [N, DIM], F16, kind="ExternalOutput").ap()

    Exp = mybir.ActivationFunctionType.Exp

    from contextlib import ExitStack

    with tile.TileContext(nc) as tc, ExitStack() as ctx:
        const = ctx.enter_context(tc.tile_pool(name="const", bufs=1))
        bk_sb = const.tile([P, E // P], F32)
        nc.sync.dma_start(out=bk_sb, in_=bk_d.rearrange("(t p) -> p t", p=P))

        persist = ctx.enter_context(tc.tile_pool(name="persist", bufs=1))
        qT = persist.tile([P, E // P, N], F16)
        kT = persist.tile([P, E // P, M], F16)
        vaug = persist.tile([P, MT, HPC, HEAD_DIM + 1], F16)
        woT = persist.tile([P, E // P, DIM], F16)
        otn2 = persist.tile([P, E // P, N], F16)

        # ones column: fill everything; v evictions overwrite cols 0:64
        nc.vector.memset(vaug, 1.0)

        bwork = ctx.enter_context(tc.tile_pool(name="bwork", bufs=4))
        maskp = ctx.enter_context(tc.tile_pool(name="maskp", bufs=3))
        rbp = ctx.enter_context(tc.tile_pool(name="rbp", bufs=2))
        dnp = ctx.enter_context(tc.tile_pool(name="dnp", bufs=2))

        def emit_scores(spool, sbufs, h, mt, exmst, mk):
            """scores -> exp -> mask for head h at m-tile mt. One
            [128, 2, 512] psum tile per (h, mt); exp over the full 1024 free
            in one ACT instr; one flat DVE mask-mul (2x_1p mode)."""
            hp, hl = divmod(h, 2)
            erow = slice(hl * HEAD_DIM, (hl + 1) * HEAD_DIM)
            st = spool.tile([P, 2, 512], F32, tag="st", name="st", bufs=sbufs)
            for chn in range(2):
                nc.tensor.matmul(
                    st[:, chn, :],
                    lhsT=kT[erow, hp, mt * P : (mt + 1) * P],
                    rhs=qT[erow, hp, chn * 512 : (chn + 1) * 512],
                    start=True,
                    stop=True,
                )
            ex = bwork.tile([P, 2, 512], F16, tag="ex", name="ex")
            nc.scalar.activation(ex, st, Exp, scale=float(SCALE))
            nc.vector.tensor_mul(exmst[:, mt, hl, :], ex, mk)

        def emit_pv(ot_ps, hp, mt, exmst, chn):
            for hl in range(2):
                h = hp * 2 + hl
                nc.tensor.matmul(
                    ot_ps[hl],
                    lhsT=vaug[:, mt, h, :],
                    rhs=exmst[:, mt, hl, chn * 512 : (chn + 1) * 512],
                    start=(mt == 0),
                    stop=(mt == MT - 1),
                )

        def normalize_pair(hp, chn, ot_ps):
            """softmax-normalize head pair hp's n-half chn straight from the
            PV psum accumulators ot_ps (list: hl -> [65, 512] psum tile).
            Denominators (psum row 64) -> sbuf park -> partition 0 via shift
            DMA -> reciprocal -> partition_broadcast; one fused mul-evict per
            head writes normalized fp16 into otn2. Odd head shifts to
            partitions 64:128 via SBUF-SBUF DMA."""
            cs = slice(chn * 512, (chn + 1) * 512)
            dpk = dnp.tile([P, 2, 512], F32, tag="dpk", name="dpk")
            for hl in range(2):
                nc.vector.tensor_copy(dpk[64:65, hl, :], ot_ps[hl][64:65, :])
            dna = dnp.tile([1, 2, 512], F32, tag="dna", name="dna")
            nc.sync.dma_start(out=dna, in_=dpk[64:65, :, :])
            rca = rbp.tile([1, 2, 512], F32, tag="rca", name="rca")
            nc.vector.reciprocal_approx_fast(out=rca, in_=dna)
            rba = rbp.tile([HEAD_DIM, 2, 512], F32, tag="rba", name="rba")
            nc.gpsimd.partition_broadcast(rba, rca)
            nc.vector.tensor_mul(
                otn2[:HEAD_DIM, hp, cs], ot_ps[0][:HEAD_DIM, :], rba[:, 0, :]
            )
            tmp = rbp.tile([HEAD_DIM, 512], F16, tag="tmp", name="tmp")
            nc.vector.tensor_mul(tmp, ot_ps[1][:HEAD_DIM, :], rba[:, 1, :])
            # partition shift 0:64 -> 64:128 via SBUF-SBUF DMA
            nc.gpsimd.dma_start(out=otn2[HEAD_DIM:P, hp, cs], in_=tmp)

        def load_mask(mt, ring):
            mk = maskp.tile([P, N], F16, tag="mk", name="mk")
            ring.dma_start(out=mk, in_=maskt_d[mt * P : (mt + 1) * P, :])
            return mk

        with tc.tile_pool(name="exmp", bufs=1) as exmp:
            # masked exp(scores) parked per m-tile; one buffer reused across
            # head pairs (WAR: stage-2 rewrites a tile only after its PV read)
            exmst = exmp.tile([P, MT, 2, N], F16)

            with tc.tile_pool(name="wctx", bufs=1) as wctx_pool:
                wkT = wctx_pool.tile([P, CT, E], F16)
                wvT = wctx_pool.tile([P, CT, E], F16)
                ctxT = wctx_pool.tile([P, CT, M], F16)

                with tc.tile_pool(name="qx", bufs=1) as qx_pool:
                    wqT = qx_pool.tile([P, CT, E], F16)
                    xT = qx_pool.tile([P, CT, N], F8)
                    # DMA rings are issue-rate bound (~0.6us/instr) and each
                    # sustains only ~140 GB/s, so the prologue spreads the
                    # critical loads across all three rings, dependency-first:
                    #   sync:   wq, then x j-chunks (Q proj gate)
                    #   scalar: wk, ctx m-quarters (K proj gate), wv, wo
                    #   gpsimd: x tail chunks
                    nc.sync.dma_start(
                        out=wqT, in_=wqT_d.rearrange("(c p) e -> p c e", p=P)
                    )
                    for jp in range(2):
                        nc.sync.dma_start(
                            out=xT[:, 2 * jp : 2 * jp + 2, :],
                            in_=xT_d[jp * 2 * P : (jp + 1) * 2 * P, :].rearrange(
                                "(c p) n -> p c n", p=P
                            ),
                        )
                    for jp in range(2, 4):
                        nc.gpsimd.dma_start(
                            out=xT[:, 2 * jp : 2 * jp + 2, :],
                            in_=xT_d[jp * 2 * P : (jp + 1) * 2 * P, :].rearrange(
                                "(c p) n -> p c n", p=P
                            ),
                        )
                    nc.scalar.dma_start(
                        out=wkT, in_=wkT_d.rearrange("(c p) e -> p c e", p=P)
                    )
                    for q in range(4):
                        for jp in range(CT // 2):
                            nc.scalar.dma_start(
                                out=ctxT[
                                    :, 2 * jp : 2 * jp + 2, q * 512 : (q + 1) * 512
                                ],
                                in_=ctxT_d[
                                    jp * 2 * P : (jp + 1) * 2 * P,
                                    q * 512 : (q + 1) * 512,
                                ].rearrange("(c p) m -> p c m", p=P),
                            )
                    nc.scalar.dma_start(
                        out=wvT, in_=wvT_d.rearrange("(c p) e -> p c e", p=P)
                    )
                    nc.scalar.dma_start(
                        out=woT, in_=woT_d.rearrange("(c p) e -> p c e", p=P)
                    )

                    # Q projection, contraction-chunk outer: the PE consumes
                    # x chunks as they land
                    qgroups = [
                        (et, chn) for et in range(E // P) for chn in range(N // 512)
                    ]
                    with tc.tile_pool(name="ppsA", bufs=1, space="PSUM") as ppsA:
                        pqs = {
                            g: ppsA.tile([P, 512], F32, tag=f"pq{i}", name=f"pq{i}")
                            for i, g in enumerate(qgroups)
                        }
                        for j in range(CT):
                            for et, chn in qgroups:
                                nc.tensor.matmul(
                                    pqs[(et, chn)],
                                    lhsT=wqT[:, j, et * P : (et + 1) * P],
                                    rhs=xT[:, j, chn * 512 : (chn + 1) * 512],
                                    start=(j == 0),
                                    stop=(j == CT - 1),
                                )
                        for et, chn in qgroups:
                            nc.vector.tensor_copy(
                                qT[:, et, chn * 512 : (chn + 1) * 512],
                                pqs[(et, chn)],
                            )

                def emit_kproj(kps, et, chm):
                    pk = kps.tile([P, 512], F32, tag="pk", name="pk")
                    for j in range(CT):
                        nc.tensor.matmul(
                            pk,
                            lhsT=wkT[:, j, et * P : (et + 1) * P],
                            rhs=ctxT[:, j, chm * 512 : (chm + 1) * 512],
                            start=(j == 0),
                            stop=(j == CT - 1),
                        )
                    nc.vector.tensor_scalar_add(
                        kT[:, et, chm * 512 : (chm + 1) * 512],
                        pk,
                        bk_sb[:, et : et + 1],
                    )

                # stage 1: scores+exp+mask pair A (ACT-bound) with the V and
                # K projections and PV-A chn0 interleaved on the PE. Pair A
                # needs only kT et0; et1 chunks are paced ahead of stage 2.
                # PSUM: sps1 4 + vps 1 + kps 1 + opsA0 2 = 8 banks.
                with (
                    tc.tile_pool(name="sps1", bufs=1, space="PSUM") as sps1,
                    tc.tile_pool(name="vps", bufs=1, space="PSUM") as vps,
                    tc.tile_pool(name="kps", bufs=1, space="PSUM") as kps,
                    tc.tile_pool(name="opsA0", bufs=1, space="PSUM") as opsA0,
                ):
                    ot_psA0 = [
                        opsA0.tile(
                            [HEAD_DIM + 1, 512], F32, tag=f"a0{i}", name=f"a0{i}"
                        )
                        for i in range(2)
                    ]
                    emit_kproj(kps, 0, 0)
                    for mt in range(MT):
                        # keep kT(et0) one chunk ahead of the scores that
                        # consume it; kT(et1) lands before stage 2
                        if mt % 2 == 0:
                            et, chm = divmod(mt // 2 + 1, M // 512)
                            if et < 2:
                                emit_kproj(kps, et, chm)
                        mk = load_mask(mt, nc.sync)
                        for hl in range(2):
                            emit_scores(sps1, 2, hl, mt, exmst, mk)
                        pv = vps.tile([P, HPC, HEAD_DIM], F32, tag="pv")
                        for j in range(CT):
                            nc.tensor.matmul(
                                pv,
                                lhsT=ctxT[:, j, mt * P : (mt + 1) * P],
                                rhs=wvT[:, j, :],
                                start=(j == 0),
                                stop=(j == CT - 1),
                            )
                        # batched strided v eviction (Pool can't read PSUM)
                        nc.vector.tensor_copy(vaug[:, mt, :, :HEAD_DIM], pv)
                        emit_pv(ot_psA0, 0, mt, exmst, 0)
                    # normalize A chn0 straight from psum; overlaps stage-2
                    # PE work (stage-2 allocations wait only on these banks)
                    normalize_pair(0, 0, ot_psA0)

            # stage 2: PV-A chn1 + scores pair B + PV-B chn0 per m-tile.
            # PSUM: sps2 4 + opsA1 2 + opsB0 2 = 8 banks (opsA0 drains into
            # the first iterations).
            with (
                tc.tile_pool(name="sps2", bufs=1, space="PSUM") as sps2,
                tc.tile_pool(name="opsA1", bufs=1, space="PSUM") as opsA1,
                tc.tile_pool(name="opsB0", bufs=1, space="PSUM") as opsB0,
            ):
                ot_psA1 = [
                    opsA1.tile(
                        [HEAD_DIM + 1, 512], F32, tag=f"a1{i}", name=f"a1{i}"
                    )
                    for i in range(2)
                ]
                ot_psB0 = [
                    opsB0.tile(
                        [HEAD_DIM + 1, 512], F32, tag=f"b0{i}", name=f"b0{i}"
                    )
                    for i in range(2)
                ]
                for mt in range(MT):
                    mk = load_mask(mt, nc.gpsimd)
                    emit_pv(ot_psA1, 0, mt, exmst, 1)
                    for hl in range(2):
                        emit_scores(sps2, 2, 2 + hl, mt, exmst, mk)
                    emit_pv(ot_psB0, 1, mt, exmst, 0)
                normalize_pair(0, 1, ot_psA1)
                normalize_pair(1, 0, ot_psB0)

            # stage 3: PV-B chn1 interleaved with the first half of the
            # output projection; ys evictions + y DMA ride the now-idle ACT
            # engine/ring.
            with (
                tc.tile_pool(name="opsB1", bufs=1, space="PSUM") as opsB1,
                tc.tile_pool(name="ypsum", bufs=3, space="PSUM") as ypsum,
                tc.tile_pool(name="ypool", bufs=3) as ypool,
            ):
                def emit_oproj(nb):
                    ys = ypool.tile([P, DIM], F16, tag="ys", name="ys")
                    for oc in range(DIM // 512):
                        yp = ypsum.tile([P, 512], F32, tag="yp", name="yp")
                        for hp in range(E // P):
                            nc.tensor.matmul(
                                yp,
                                lhsT=otn2[:, hp, nb * P : (nb + 1) * P],
                                rhs=woT[:, hp, oc * 512 : (oc + 1) * 512],
                                start=(hp == 0),
                                stop=(hp == E // P - 1),
                            )
                        nc.scalar.copy(ys[:, oc * 512 : (oc + 1) * 512], yp)
                    nc.scalar.dma_start(out=y_d[nb * P : (nb + 1) * P, :], in_=ys)

                ot_psB1 = [
                    opsB1.tile(
                        [HEAD_DIM + 1, 512], F32, tag=f"b1{i}", name=f"b1{i}"
                    )
                    for i in range(2)
                ]
                for mt in range(MT):
                    emit_pv(ot_psB1, 1, mt, exmst, 1)
                    if mt >= 9 and mt % 2 == 1:
                        emit_oproj((mt - 9) // 2)
                emit_oproj(3)
                normalize_pair(1, 1, ot_psB1)
                for nb in range(4, N // P):
                    emit_oproj(nb)

    nc.compile()
    return nc


_NC_CACHE = []


def _get_nc():
    if not _NC_CACHE:
        _NC_CACHE.append(build_program())
    return _NC_CACHE[0]


def make_in_maps(x, context, mask, Wq, Wkv, b_kv, Wo):
    f16 = ml_dtypes.bfloat16
    f8 = ml_dtypes.float8_e4m3
    x = np.asarray(x, dtype=np.float32)
    context = np.asarray(context, dtype=np.float32)
    mask = np.asarray(mask)
    Wq = np.asarray(Wq, dtype=np.float32)
    Wkv = np.asarray(Wkv, dtype=np.float32)
    b_kv = np.asarray(b_kv, dtype=np.float32)
    Wo = np.asarray(Wo, dtype=np.float32)

    in_maps = []
    for b in range(B):
        xtb = np.ascontiguousarray(x[b].T).astype(f8)
        ctb = np.ascontiguousarray(context[b].T).astype(f16)
        mtb = np.ascontiguousarray(mask[b].T).astype(f16)
        for g in range(NUM_HEADS // HPC):
            sl = slice(E * g, E * (g + 1))
            in_maps.append(
                {
                    "xT": xtb,
                    "ctxT": ctb,
                    "maskt": mtb,
                    "wqT": np.ascontiguousarray(Wq[sl].T).astype(f16),
                    "wkT": np.ascontiguousarray(Wkv[sl].T).astype(f16),
                    "wvT": np.ascontiguousarray(
                        Wkv[DIM + E * g : DIM + E * (g + 1)].T
                    ).astype(f16),
                    "woT": np.ascontiguousarray(Wo[:, sl].T).astype(f16),
                    "bk": np.ascontiguousarray(b_kv[sl]),
                }
            )
    return in_maps


def combine_outputs(ys, b_kv, Wo):
    """ys: list of 8 per-core partial outputs [N, DIM], core order (b, g)."""
    b_v = np.asarray(b_kv, dtype=np.float32)[DIM:]
    ybias = np.asarray(Wo, dtype=np.float32) @ b_v  # [DIM]
    out = np.empty((B, N, DIM), dtype=np.float32)
    G = NUM_HEADS // HPC
    for b in range(B):
        acc = np.asarray(ys[G * b], dtype=np.float32)
        for g in range(1, G):
            acc = acc + np.asarray(ys[G * b + g], dtype=np.float32)
        out[b] = acc + ybias[None, :]
    return out


def kernel(x, context, mask, Wq, Wkv, b_kv, Wo):
    nc = _get_nc()
    in_maps = make_in_maps(x, context, mask, Wq, Wkv, b_kv, Wo)
    res = run_bass_kernel_spmd(nc, in_maps, core_ids=list(range(8)))
    ys = [m["y"] for m in res.results]
    return combine_outputs(ys, b_kv, Wo)


# revision 17
# speedup vs baseline: 1.2025x; 1.0043x over previous
"""CrossAttention Trainium2 kernel (8 NeuronCores, SPMD).

Sharding: data-parallel over batch B=2, tensor-parallel over the 16 heads in
4 groups of 4 heads -> 8 cores, one (batch, head-group) pair each. Each core
computes its 4 heads' Q/K/V projections, masked softmax cross-attention, and
its partial output projection y_g = softmax(q k^T * scale) v @ Wo[:, g].T.
The host sums the 4 partial outputs per batch (the Wo row-split all-reduce,
done at unshard time) and adds the v-bias term Wo @ b_v, which is constant
across rows and factors out of the attention (softmax rows sum to 1).

Numerics: fp16 matmuls with fp32 PSUM accumulation (fp16 costs the same as
bf16 on every engine and carries 3 extra mantissa bits; all tensors here are
comfortably inside fp16 range). x travels as fp8e4m3, halving its DMA
footprint; the quantization only perturbs q and thus the softmax weights,
damped by the small score scale. The PE is row-stream bound (cycles = moving
free size regardless of contraction width or dtype), so fp8 DoubleRow
matmuls are NOT used: measured on hardware they process 2x the moving rows
for the same output, a net loss.

Layout: activations and weights arrive contraction-major (pre-transposed on
the host) so every device DMA is a contiguous row load. Scores are computed
transposed: ST[m, n] per head, so the PV matmul contracts over m directly,
and an appended ones-column on the V stationary operand yields the softmax
denominator for free. exp() is unnormalized (no max subtraction; scores are
bounded); mask zeros are applied multiplicatively after exp (DVE, 2x mode).

Normalization happens straight out of the PV PSUM accumulators (no fp32
park): denominator row 64 -> partition 0 via a tiny SBUF shift DMA,
reciprocal (DVE), partition_broadcast (Pool), then one fused multiply-evict
per head writes normalized fp16 into otn2. Odd heads reach partitions 64:128
via an SBUF-SBUF shift DMA.

Schedule: the kernel is PE-bound (~230K PE cycles vs ~72us of exp on ACT),
so PV work is pulled forward under the exp stream instead of trailing it:
  stage 1: scores+exp+mask pair A with the V and K projections and PV-A
           chn0 interleaved on the PE.
  stage 2: PV-A chn1 + scores pair B + PV-B chn0 per m-tile.
  stage 3: PV-B chn1 overlapped with the first half of the output
           projection; ys evictions + y DMA ride the ACT engine/ring, idle
           once the exp stream has drained.
Input DMAs are spread across all three rings (sync/scalar/gpsimd) with
x j-chunked so the Q projection starts as early as possible.
"""

import numpy as np
import ml_dtypes

import concourse.bass as bass
import concourse.bacc as bacc
import concourse.mybir as mybir
import concourse.tile as tile
from concourse.bass_utils import run_bass_kernel_spmd

DIM = 1024
HEAD_DIM = 64
NUM_HEADS = 16
SCALE = HEAD_DIM**-0.5
B, N, M = 2, 1024, 2048
HPC = 4  # heads per core
E = HPC * HEAD_DIM  # 256: per-core projection width
P = 128
F32 = mybir.dt.float32
F16 = mybir.dt.bfloat16  # bf16: measured ~20% faster per matmul than fp16 on HW
F8 = mybir.dt.float8e4
CT = DIM // P  # 8 contraction tiles
MT = M // P  # 16 m tiles


def build_program():
    nc = bacc.Bacc("TRN2", target_bir_lowering=False, debug=False, num_devices=8)

    # all activation/weight shards arrive contraction-major (pre-transposed)
    xT_d = nc.dram_tensor("xT", [DIM, N], F8, kind="ExternalInput").ap()
    ctxT_d = nc.dram_tensor("ctxT", [DIM, M], F16, kind="ExternalInput").ap()
    maskt_d = nc.dram_tensor("maskt", [M, N], F16, kind="ExternalInput").ap()
    wqT_d = nc.dram_tensor("wqT", [DIM, E], F16, kind="ExternalInput").ap()
    wkT_d = nc.dram_tensor("wkT", [DIM, E], F16, kind="ExternalInput").ap()
    wvT_d = nc.dram_tensor("wvT", [DIM, E], F16, kind="ExternalInput").ap()
    woT_d = nc.dram_tensor("woT", [E, DIM], F16, kind="ExternalInput").ap()
    bk_d = nc.dram_tensor("bk", [E], F32, kind="ExternalInput").ap()
    y_d = nc.dram_tensor("y", [N, DIM], F16, kind="ExternalOutput").ap()

    Exp = mybir.ActivationFunctionType.Exp

    from contextlib import ExitStack

    with tile.TileContext(nc) as tc, ExitStack() as ctx:
        const = ctx.enter_context(tc.tile_pool(name="const", bufs=1))
        bk_sb = const.tile([P, E // P], F32)
        nc.sync.dma_start(out=bk_sb, in_=bk_d.rearrange("(t p) -> p t", p=P))

        persist = ctx.enter_context(tc.tile_pool(name="persist", bufs=1))
        qT = persist.tile([P, E // P, N], F16)
        kT = persist.tile([P, E // P, M], F16)
        vaug = persist.tile([P, MT, HPC, HEAD_DIM + 1], F16)
        woT = persist.tile([P, E // P, DIM], F16)
        otn2 = persist.tile([P, E // P, N], F16)

        # ones column: fill everything; v evictions overwrite cols 0:64
        nc.vector.memset(vaug, 1.0)

        bwork = ctx.enter_context(tc.tile_pool(name="bwork", bufs=4))
        maskp = ctx.enter_context(tc.tile_pool(name="maskp", bufs=3))
        rbp = ctx.enter_context(tc.tile_pool(name="rbp", bufs=2))
        dnp = ctx.enter_context(tc.tile_pool(name="dnp", bufs=2))

        def emit_scores(spool, sbufs, h, mt, exmst, mk):
            """scores -> exp -> mask for head h at m-tile mt. One
            [128, 2, 512] psum tile per (h, mt); exp over the full 1024 free
            in one ACT instr; one flat DVE mask-mul (2x_1p mode)."""
            hp, hl = divmod(h, 2)
            erow = slice(hl * HEAD_DIM, (hl + 1) * HEAD_DIM)
            st = spool.tile([P, 2, 512], F32, tag="st", name="st", bufs=sbufs)
            for chn in range(2):
                nc.tensor.matmul(
                    st[:, chn, :],
                    lhsT=kT[erow, hp, mt * P : (mt + 1) * P],
                    rhs=qT[erow, hp, chn * 512 : (chn + 1) * 512],
                    start=True,
                    stop=True,
                )
            ex = bwork.tile([P, 2, 512], F16, tag="ex", name="ex")
            nc.scalar.activation(ex, st, Exp, scale=float(SCALE))
            nc.vector.tensor_mul(exmst[:, mt, hl, :], ex, mk)

        def emit_pv(ot_ps, hp, mt, exmst, chn):
            # NOTE: callers emit PV a few m-tiles BEHIND the exmst producer
            # so the in-order PE queue never waits on a fresh exp->mask chain
            for hl in range(2):
                h = hp * 2 + hl
                nc.tensor.matmul(
                    ot_ps[hl],
                    lhsT=vaug[:, mt, h, :],
                    rhs=exmst[:, mt, hl, chn * 512 : (chn + 1) * 512],
                    start=(mt == 0),
                    stop=(mt == MT - 1),
                )

        def normalize_pair(hp, chn, ot_ps):
            """softmax-normalize head pair hp's n-half chn straight from the
            PV psum accumulators ot_ps (list: hl -> [65, 512] psum tile).
            Denominators (psum row 64) -> sbuf park -> partition 0 via shift
            DMA -> reciprocal -> partition_broadcast; one fused mul-evict per
            head writes normalized fp16 into otn2. Odd head shifts to
            partitions 64:128 via SBUF-SBUF DMA."""
            cs = slice(chn * 512, (chn + 1) * 512)
            dpk = dnp.tile([P, 2, 512], F32, tag="dpk", name="dpk")
            for hl in range(2):
                nc.vector.tensor_copy(dpk[64:65, hl, :], ot_ps[hl][64:65, :])
            dna = dnp.tile([1, 2, 512], F32, tag="dna", name="dna")
            nc.sync.dma_start(out=dna, in_=dpk[64:65, :, :])
            rca = rbp.tile([1, 2, 512], F32, tag="rca", name="rca")
            nc.vector.reciprocal_approx_fast(out=rca, in_=dna)
            rba = rbp.tile([HEAD_DIM, 2, 512], F32, tag="rba", name="rba")
            nc.gpsimd.partition_broadcast(rba, rca)
            nc.vector.tensor_mul(
                otn2[:HEAD_DIM, hp, cs], ot_ps[0][:HEAD_DIM, :], rba[:, 0, :]
            )
            tmp = rbp.tile([HEAD_DIM, 512], F16, tag="tmp", name="tmp")
            nc.vector.tensor_mul(tmp, ot_ps[1][:HEAD_DIM, :], rba[:, 1, :])
            # partition shift 0:64 -> 64:128 via SBUF-SBUF DMA
            nc.gpsimd.dma_start(out=otn2[HEAD_DIM:P, hp, cs], in_=tmp)

        def load_mask(mt, ring):
            mk = maskp.tile([P, N], F16, tag="mk", name="mk")
            ring.dma_start(out=mk, in_=maskt_d[mt * P : (mt + 1) * P, :])
            return mk

        with tc.tile_pool(name="exmp", bufs=1) as exmp:
            # masked exp(scores) parked per m-tile; one buffer reused across
            # head pairs (WAR: stage-2 rewrites a tile only after its PV read)
            exmst = exmp.tile([P, MT, 2, N], F16)

            with tc.tile_pool(name="wctx", bufs=1) as wctx_pool:
                wkT = wctx_pool.tile([P, CT, E], F16)
                wvT = wctx_pool.tile([P, CT, E], F16)
                ctxT = wctx_pool.tile([P, CT, M], F16)

                with tc.tile_pool(name="qx", bufs=1) as qx_pool:
                    wqT = qx_pool.tile([P, CT, E], F16)
                    xT = qx_pool.tile([P, CT, N], F8)
                    # DMA rings are issue-rate bound (~0.6us/instr) and each
                    # sustains only ~140 GB/s, so the prologue spreads the
                    # critical loads across all three rings, dependency-first
                    # and j-granular so Q proj j=0 starts as early as possible:
                    #   sync:   wq half 0, x j0-3 (Q proj gate)
                    #   gpsimd: wq half 1, x j4-7
                    #   scalar: wk, ctx m-quarters (K proj gate), wv, wo
                    nc.sync.dma_start(
                        out=wqT[:, :4, :],
                        in_=wqT_d[: DIM // 2, :].rearrange("(c p) e -> p c e", p=P),
                    )
                    nc.gpsimd.dma_start(
                        out=wqT[:, 4:, :],
                        in_=wqT_d[DIM // 2 :, :].rearrange("(c p) e -> p c e", p=P),
                    )
                    for jp in range(2):
                        nc.sync.dma_start(
                            out=xT[:, 2 * jp : 2 * jp + 2, :],
                            in_=xT_d[jp * 2 * P : (jp + 1) * 2 * P, :].rearrange(
                                "(c p) n -> p c n", p=P
                            ),
                        )
                    for jp in range(2, 4):
                        nc.gpsimd.dma_start(
                            out=xT[:, 2 * jp : 2 * jp + 2, :],
                            in_=xT_d[jp * 2 * P : (jp + 1) * 2 * P, :].rearrange(
                                "(c p) n -> p c n", p=P
                            ),
                        )
                    nc.scalar.dma_start(
                        out=wkT, in_=wkT_d.rearrange("(c p) e -> p c e", p=P)
                    )
                    for q in range(4):
                        for jp in range(CT // 2):
                            nc.scalar.dma_start(
                                out=ctxT[
                                    :, 2 * jp : 2 * jp + 2, q * 512 : (q + 1) * 512
                                ],
                                in_=ctxT_d[
                                    jp * 2 * P : (jp + 1) * 2 * P,
                                    q * 512 : (q + 1) * 512,
                                ].rearrange("(c p) m -> p c m", p=P),
                            )
                    nc.scalar.dma_start(
                        out=wvT, in_=wvT_d.rearrange("(c p) e -> p c e", p=P)
                    )
                    nc.scalar.dma_start(
                        out=woT, in_=woT_d.rearrange("(c p) e -> p c e", p=P)
                    )

                    # Q projection, contraction-chunk outer: the PE consumes
                    # x chunks as they land
                    qgroups = [
                        (et, chn) for et in range(E // P) for chn in range(N // 512)
                    ]
                    with tc.tile_pool(name="ppsA", bufs=1, space="PSUM") as ppsA:
                        pqs = {
                            g: ppsA.tile([P, 512], F32, tag=f"pq{i}", name=f"pq{i}")
                            for i, g in enumerate(qgroups)
                        }
                        for j in range(CT):
                            for et, chn in qgroups:
                                nc.tensor.matmul(
                                    pqs[(et, chn)],
                                    lhsT=wqT[:, j, et * P : (et + 1) * P],
                                    rhs=xT[:, j, chn * 512 : (chn + 1) * 512],
                                    start=(j == 0),
                                    stop=(j == CT - 1),
                                )
                        for et, chn in qgroups:
                            nc.vector.tensor_copy(
                                qT[:, et, chn * 512 : (chn + 1) * 512],
                                pqs[(et, chn)],
                            )

                def emit_kproj(kps, et, chm):
                    pk = kps.tile([P, 512], F32, tag="pk", name="pk")
                    for j in range(CT):
                        nc.tensor.matmul(
                            pk,
                            lhsT=wkT[:, j, et * P : (et + 1) * P],
                            rhs=ctxT[:, j, chm * 512 : (chm + 1) * 512],
                            start=(j == 0),
                            stop=(j == CT - 1),
                        )
                    nc.vector.tensor_scalar_add(
                        kT[:, et, chm * 512 : (chm + 1) * 512],
                        pk,
                        bk_sb[:, et : et + 1],
                    )

                # stage 1: scores+exp+mask pair A (ACT-bound) with the V and
                # K projections and PV-A chn0 interleaved on the PE. Pair A
                # needs only kT et0; et1 chunks are paced ahead of stage 2.
                # PSUM: sps1 4 + vps 1 + kps 1 + opsA0 2 = 8 banks.
                with (
                    tc.tile_pool(name="sps1", bufs=1, space="PSUM") as sps1,
                    tc.tile_pool(name="vps", bufs=1, space="PSUM") as vps,
                    tc.tile_pool(name="kps", bufs=1, space="PSUM") as kps,
                    tc.tile_pool(name="opsA0", bufs=1, space="PSUM") as opsA0,
                ):
                    ot_psA0 = [
                        opsA0.tile(
                            [HEAD_DIM + 1, 512], F32, tag=f"a0{i}", name=f"a0{i}"
                        )
                        for i in range(2)
                    ]
                    emit_kproj(kps, 0, 0)
                    LAG = 2  # PV trails its exmst producer by LAG m-tiles
                    for mt in range(MT):
                        # keep kT(et0) one chunk ahead of the scores that
                        # consume it; kT(et1) lands before stage 2
                        if mt % 2 == 0:
                            et, chm = divmod(mt // 2 + 1, M // 512)
                            if et < 2:
                                emit_kproj(kps, et, chm)
                        mk = load_mask(mt, nc.sync)
                        for hl in range(2):
                            emit_scores(sps1, 2, hl, mt, exmst, mk)
                        pv = vps.tile([P, HPC, HEAD_DIM], F32, tag="pv")
                        for j in range(CT):
                            nc.tensor.matmul(
                                pv,
                                lhsT=ctxT[:, j, mt * P : (mt + 1) * P],
                                rhs=wvT[:, j, :],
                                start=(j == 0),
                                stop=(j == CT - 1),
                            )
                        # batched strided v eviction (Pool can't read PSUM)
                        nc.vector.tensor_copy(vaug[:, mt, :, :HEAD_DIM], pv)
                        if mt >= LAG:
                            emit_pv(ot_psA0, 0, mt - LAG, exmst, 0)
                    for mt in range(MT - LAG, MT):
                        emit_pv(ot_psA0, 0, mt, exmst, 0)
                    # normalize A chn0 straight from psum; overlaps stage-2
                    # PE work (stage-2 allocations wait only on these banks)
                    normalize_pair(0, 0, ot_psA0)

            # stage 2: PV-A chn1 + scores pair B + PV-B chn0 per m-tile.
            # PSUM: sps2 4 + opsA1 2 + opsB0 2 = 8 banks (opsA0 drains into
            # the first iterations).
            with (
                tc.tile_pool(name="sps2", bufs=1, space="PSUM") as sps2,
                tc.tile_pool(name="opsA1", bufs=1, space="PSUM") as opsA1,
                tc.tile_pool(name="opsB0", bufs=1, space="PSUM") as opsB0,
            ):
                ot_psA1 = [
                    opsA1.tile(
                        [HEAD_DIM + 1, 512], F32, tag=f"a1{i}", name=f"a1{i}"
                    )
                    for i in range(2)
                ]
                ot_psB0 = [
                    opsB0.tile(
                        [HEAD_DIM + 1, 512], F32, tag=f"b0{i}", name=f"b0{i}"
                    )
                    for i in range(2)
                ]
                LAG = 2
                mk_q = [load_mask(mt, nc.gpsimd) for mt in range(LAG)]
                for mt in range(MT):
                    if mt + LAG < MT:
                        mk_q.append(load_mask(mt + LAG, nc.gpsimd))
                    mk = mk_q.pop(0)
                    # PV-A1 reads stage-1 exmst (long ready, no stall); it
                    # must precede this mt's scores-B overwrite (WAR)
                    emit_pv(ot_psA1, 0, mt, exmst, 1)
                    for hl in range(2):
                        emit_scores(sps2, 2, 2 + hl, mt, exmst, mk)
                    if mt >= LAG:
                        emit_pv(ot_psB0, 1, mt - LAG, exmst, 0)
                for mt in range(MT - LAG, MT):
                    emit_pv(ot_psB0, 1, mt, exmst, 0)
                normalize_pair(0, 1, ot_psA1)
                normalize_pair(1, 0, ot_psB0)

            # stage 3: PV-B chn1 interleaved with the first half of the
            # output projection; ys evictions + y DMA ride the now-idle ACT
            # engine/ring.
            with (
                tc.tile_pool(name="opsB1", bufs=1, space="PSUM") as opsB1,
                tc.tile_pool(name="ypsum", bufs=3, space="PSUM") as ypsum,
                tc.tile_pool(name="ypool", bufs=3) as ypool,
            ):
                def emit_oproj(nb):
                    ys = ypool.tile([P, DIM], F16, tag="ys", name="ys")
                    for oc in range(DIM // 512):
                        yp = ypsum.tile([P, 512], F32, tag="yp", name="yp")
                        for hp in range(E // P):
                            nc.tensor.matmul(
                                yp,
                                lhsT=otn2[:, hp, nb * P : (nb + 1) * P],
                                rhs=woT[:, hp, oc * 512 : (oc + 1) * 512],
                                start=(hp == 0),
                                stop=(hp == E // P - 1),
                            )
                        nc.scalar.copy(ys[:, oc * 512 : (oc + 1) * 512], yp)
                        # per-half writeback, alternating rings: drains the
                        # tail ~2x faster than one whole-row DMA at the end
                        ring = nc.scalar if oc == 0 else nc.sync
                        ring.dma_start(
                            out=y_d[nb * P : (nb + 1) * P, oc * 512 : (oc + 1) * 512],
                            in_=ys[:, oc * 512 : (oc + 1) * 512],
                        )

                ot_psB1 = [
                    opsB1.tile(
                        [HEAD_DIM + 1, 512], F32, tag=f"b1{i}", name=f"b1{i}"
                    )
                    for i in range(2)
                ]
                for mt in range(MT):
                    emit_pv(ot_psB1, 1, mt, exmst, 1)
                    if mt >= 9 and mt % 2 == 1:
                        emit_oproj((mt - 9) // 2)
                emit_oproj(3)
                normalize_pair(1, 1, ot_psB1)
                for nb in range(4, N // P):
                    emit_oproj(nb)

    nc.compile()
    return nc


_NC_CACHE = []


def _get_nc():
    if not _NC_CACHE:
        _NC_CACHE.append(build_program())
    return _NC_CACHE[0]


def make_in_maps(x, context, mask, Wq, Wkv, b_kv, Wo):
    f16 = ml_dtypes.bfloat16
    f8 = ml_dtypes.float8_e4m3
    x = np.asarray(x, dtype=np.float32)
    context = np.asarray(context, dtype=np.float32)
    mask = np.asarray(mask)
    Wq = np.asarray(Wq, dtype=np.float32)
    Wkv = np.asarray(Wkv, dtype=np.float32)
    b_kv = np.asarray(b_kv, dtype=np.float32)
    Wo = np.asarray(Wo, dtype=np.float32)

    in_maps = []
    for b in range(B):
        xtb = np.ascontiguousarray(x[b].T).astype(f8)
        ctb = np.ascontiguousarray(context[b].T).astype(f16)
        mtb = np.ascontiguousarray(mask[b].T).astype(f16)
        for g in range(NUM_HEADS // HPC):
            sl = slice(E * g, E * (g + 1))
            in_maps.append(
                {
                    "xT": xtb,
                    "ctxT": ctb,
                    "maskt": mtb,
                    "wqT": np.ascontiguousarray(Wq[sl].T).astype(f16),
                    "wkT": np.ascontiguousarray(Wkv[sl].T).astype(f16),
                    "wvT": np.ascontiguousarray(
                        Wkv[DIM + E * g : DIM + E * (g + 1)].T
                    ).astype(f16),
                    "woT": np.ascontiguousarray(Wo[:, sl].T).astype(f16),
                    "bk": np.ascontiguousarray(b_kv[sl]),
                }
            )
    return in_maps


def combine_outputs(ys, b_kv, Wo):
    """ys: list of 8 per-core partial outputs [N, DIM], core order (b, g)."""
    b_v = np.asarray(b_kv, dtype=np.float32)[DIM:]
    ybias = np.asarray(Wo, dtype=np.float32) @ b_v  # [DIM]
    out = np.empty((B, N, DIM), dtype=np.float32)
    G = NUM_HEADS // HPC
    for b in range(B):
        acc = np.asarray(ys[G * b], dtype=np.float32)
        for g in range(1, G):
            acc = acc + np.asarray(ys[G * b + g], dtype=np.float32)
        out[b] = acc + ybias[None, :]
    return out


def kernel(x, context, mask, Wq, Wkv, b_kv, Wo):
    nc = _get_nc()
    in_maps = make_in_maps(x, context, mask, Wq, Wkv, b_kv, Wo)
    res = run_bass_kernel_spmd(nc, in_maps, core_ids=list(range(8)))
    ys = [m["y"] for m in res.results]
    return combine_outputs(ys, b_kv, Wo)


# revision 19
# speedup vs baseline: 1.2877x; 1.0709x over previous
"""CrossAttention Trainium2 kernel (8 NeuronCores, SPMD).

Sharding: data-parallel over batch B=2, tensor-parallel over the 16 heads in
4 groups of 4 heads -> 8 cores, one (batch, head-group) pair each. Each core
computes its 4 heads' Q/K/V projections, masked softmax cross-attention, and
its partial output projection y_g = softmax(q k^T * scale) v @ Wo[:, g].T.
The host sums the 4 partial outputs per batch (the Wo row-split all-reduce,
done at unshard time) and adds the v-bias term Wo @ b_v, which is constant
across rows and factors out of the attention (softmax rows sum to 1).

Numerics: fp16 matmuls with fp32 PSUM accumulation (fp16 costs the same as
bf16 on every engine and carries 3 extra mantissa bits; all tensors here are
comfortably inside fp16 range). x travels as fp8e4m3, halving its DMA
footprint; the quantization only perturbs q and thus the softmax weights,
damped by the small score scale. The PE is row-stream bound (cycles = moving
free size regardless of contraction width or dtype), so fp8 DoubleRow
matmuls are NOT used: measured on hardware they process 2x the moving rows
for the same output, a net loss.

Layout: activations and weights arrive contraction-major (pre-transposed on
the host) so every device DMA is a contiguous row load. Scores are computed
transposed: ST[m, n] per head, so the PV matmul contracts over m directly,
and an appended ones-column on the V stationary operand yields the softmax
denominator for free. exp() is unnormalized (no max subtraction; scores are
bounded); mask zeros are applied multiplicatively after exp (DVE, 2x mode).

Normalization happens straight out of the PV PSUM accumulators (no fp32
park): denominator row 64 -> partition 0 via a tiny SBUF shift DMA,
reciprocal (DVE), partition_broadcast (Pool), then one fused multiply-evict
per head writes normalized fp16 into otn2. Odd heads reach partitions 64:128
via an SBUF-SBUF shift DMA.

Schedule: the kernel is PE-bound (~230K PE cycles vs ~72us of exp on ACT),
so PV work is pulled forward under the exp stream instead of trailing it:
  stage 1: scores+exp+mask pair A with the V and K projections and PV-A
           chn0 interleaved on the PE.
  stage 2: PV-A chn1 + scores pair B + PV-B chn0 per m-tile.
  stage 3: PV-B chn1 overlapped with the first half of the output
           projection; ys evictions + y DMA ride the ACT engine/ring, idle
           once the exp stream has drained.
Input DMAs are spread across all three rings (sync/scalar/gpsimd) with
x j-chunked so the Q projection starts as early as possible.
"""

import numpy as np
import ml_dtypes

import concourse.bass as bass
import concourse.bacc as bacc
import concourse.mybir as mybir
import concourse.tile as tile
from concourse.bass_utils import run_bass_kernel_spmd

DIM = 1024
HEAD_DIM = 64
NUM_HEADS = 16
SCALE = HEAD_DIM**-0.5
B, N, M = 2, 1024, 2048
HPC = 4  # heads per core
E = HPC * HEAD_DIM  # 256: per-core projection width
P = 128
F32 = mybir.dt.float32
F16 = mybir.dt.bfloat16  # bf16: measured ~20% faster per matmul than fp16 on HW
F8 = mybir.dt.float8e4
CT = DIM // P  # 8 contraction tiles
MT = M // P  # 16 m tiles


def build_program():
    nc = bacc.Bacc("TRN2", target_bir_lowering=False, debug=False, num_devices=8)

    # all activation/weight shards arrive contraction-major (pre-transposed)
    xT_d = nc.dram_tensor("xT", [DIM, N], F8, kind="ExternalInput").ap()
    ctxT_d = nc.dram_tensor("ctxT", [DIM, M], F16, kind="ExternalInput").ap()
    maskt_d = nc.dram_tensor("maskt", [M, N], F16, kind="ExternalInput").ap()
    wqT_d = nc.dram_tensor("wqT", [DIM, E], F16, kind="ExternalInput").ap()
    wkT_d = nc.dram_tensor("wkT", [DIM, E], F16, kind="ExternalInput").ap()
    wvT_d = nc.dram_tensor("wvT", [DIM, E], F16, kind="ExternalInput").ap()
    woT_d = nc.dram_tensor("woT", [E, DIM], F16, kind="ExternalInput").ap()
    bk_d = nc.dram_tensor("bk", [E], F32, kind="ExternalInput").ap()
    y_d = nc.dram_tensor("y", [N, DIM], F16, kind="ExternalOutput").ap()

    Exp = mybir.ActivationFunctionType.Exp

    from contextlib import ExitStack

    with tile.TileContext(nc) as tc, ExitStack() as ctx:
        const = ctx.enter_context(tc.tile_pool(name="const", bufs=1))
        bk_sb = const.tile([P, E // P], F32)
        nc.sync.dma_start(out=bk_sb, in_=bk_d.rearrange("(t p) -> p t", p=P))

        persist = ctx.enter_context(tc.tile_pool(name="persist", bufs=1))
        qT = persist.tile([P, E // P, N], F16)
        kT = persist.tile([P, E // P, M], F16)
        vaug = persist.tile([P, MT, HPC, HEAD_DIM + 1], F16)
        woT = persist.tile([P, E // P, DIM], F16)
        otn2 = persist.tile([P, E // P, N], F16)

        # ones column: fill everything; v evictions overwrite cols 0:64
        nc.vector.memset(vaug, 1.0)

        bwork = ctx.enter_context(tc.tile_pool(name="bwork", bufs=4))
        maskp = ctx.enter_context(tc.tile_pool(name="maskp", bufs=3))
        rbp = ctx.enter_context(tc.tile_pool(name="rbp", bufs=2))
        dnp = ctx.enter_context(tc.tile_pool(name="dnp", bufs=2))

        def emit_scores(spool, sbufs, h, mt, exmst, mk):
            """scores -> exp -> mask for head h at m-tile mt. One
            [128, 2, 512] psum tile per (h, mt); exp over the full 1024 free
            in one ACT instr; one flat DVE mask-mul (2x_1p mode)."""
            hp, hl = divmod(h, 2)
            erow = slice(hl * HEAD_DIM, (hl + 1) * HEAD_DIM)
            st = spool.tile([P, 2, 512], F32, tag="st", name="st", bufs=sbufs)
            for chn in range(2):
                nc.tensor.matmul(
                    st[:, chn, :],
                    lhsT=kT[erow, hp, mt * P : (mt + 1) * P],
                    rhs=qT[erow, hp, chn * 512 : (chn + 1) * 512],
                    start=True,
                    stop=True,
                )
            ex = bwork.tile([P, 2, 512], F16, tag="ex", name="ex")
            nc.scalar.activation(ex, st, Exp, scale=float(SCALE))
            nc.vector.tensor_mul(exmst[:, mt, hl, :], ex, mk)

        def emit_pv(ot_ps, hp, mt, exmst, chn):
            # NOTE: callers emit PV a few m-tiles BEHIND the exmst producer
            # so the in-order PE queue never waits on a fresh exp->mask chain
            for hl in range(2):
                h = hp * 2 + hl
                nc.tensor.matmul(
                    ot_ps[hl],
                    lhsT=vaug[:, mt, h, :],
                    rhs=exmst[:, mt, hl, chn * 512 : (chn + 1) * 512],
                    start=(mt == 0),
                    stop=(mt == MT - 1),
                )

        def normalize_pair(hp, chn, ot_ps):
            """softmax-normalize head pair hp's n-half chn straight from the
            PV psum accumulators ot_ps (list: hl -> [65, 512] psum tile).
            Denominators (psum row 64) -> sbuf park -> partition 0 via shift
            DMA -> reciprocal -> partition_broadcast; one fused mul-evict per
            head writes normalized fp16 into otn2. Odd head shifts to
            partitions 64:128 via SBUF-SBUF DMA."""
            cs = slice(chn * 512, (chn + 1) * 512)
            dpk = dnp.tile([P, 2, 512], F32, tag="dpk", name="dpk")
            for hl in range(2):
                nc.vector.tensor_copy(dpk[64:65, hl, :], ot_ps[hl][64:65, :])
            dna = dnp.tile([1, 2, 512], F32, tag="dna", name="dna")
            nc.sync.dma_start(out=dna, in_=dpk[64:65, :, :])
            rca = rbp.tile([1, 2, 512], F32, tag="rca", name="rca")
            nc.vector.reciprocal_approx_fast(out=rca, in_=dna)
            rba = rbp.tile([HEAD_DIM, 2, 512], F32, tag="rba", name="rba")
            nc.gpsimd.partition_broadcast(rba, rca)
            nc.vector.tensor_mul(
                otn2[:HEAD_DIM, hp, cs], ot_ps[0][:HEAD_DIM, :], rba[:, 0, :]
            )
            tmp = rbp.tile([HEAD_DIM, 512], F16, tag="tmp", name="tmp")
            nc.vector.tensor_mul(tmp, ot_ps[1][:HEAD_DIM, :], rba[:, 1, :])
            # partition shift 0:64 -> 64:128 via SBUF-SBUF DMA
            nc.gpsimd.dma_start(out=otn2[HEAD_DIM:P, hp, cs], in_=tmp)

        def load_mask(mt, ring):
            mk = maskp.tile([P, N], F16, tag="mk", name="mk")
            ring.dma_start(out=mk, in_=maskt_d[mt * P : (mt + 1) * P, :])
            return mk

        with tc.tile_pool(name="exmp", bufs=1) as exmp:
            # masked exp(scores) parked per m-tile; one buffer reused across
            # head pairs (WAR: stage-2 rewrites a tile only after its PV read)
            exmst = exmp.tile([P, MT, 2, N], F16)

            with tc.tile_pool(name="wctx", bufs=1) as wctx_pool:
                wkT = wctx_pool.tile([P, CT, E], F16)
                wvT = wctx_pool.tile([P, CT, E], F16)
                ctxT = wctx_pool.tile([P, CT, M], F16)

                with tc.tile_pool(name="qx", bufs=1) as qx_pool:
                    wqT = qx_pool.tile([P, CT, E], F16)
                    xT = qx_pool.tile([P, CT, N], F8)
                    # DMA rings are issue-rate bound (~0.6us/instr) and each
                    # sustains only ~140 GB/s, so the prologue spreads the
                    # critical loads across all three rings, dependency-first
                    # and j-granular so Q proj j=0 starts as early as possible:
                    #   sync:   wq half 0, x j0-3 (Q proj gate)
                    #   gpsimd: wq half 1, x j4-7
                    #   scalar: wk, ctx m-quarters (K proj gate), wv, wo
                    nc.sync.dma_start(
                        out=wqT[:, :4, :],
                        in_=wqT_d[: DIM // 2, :].rearrange("(c p) e -> p c e", p=P),
                    )
                    nc.gpsimd.dma_start(
                        out=wqT[:, 4:, :],
                        in_=wqT_d[DIM // 2 :, :].rearrange("(c p) e -> p c e", p=P),
                    )
                    for jp in range(2):
                        nc.sync.dma_start(
                            out=xT[:, 2 * jp : 2 * jp + 2, :],
                            in_=xT_d[jp * 2 * P : (jp + 1) * 2 * P, :].rearrange(
                                "(c p) n -> p c n", p=P
                            ),
                        )
                    for jp in range(2, 4):
                        nc.gpsimd.dma_start(
                            out=xT[:, 2 * jp : 2 * jp + 2, :],
                            in_=xT_d[jp * 2 * P : (jp + 1) * 2 * P, :].rearrange(
                                "(c p) n -> p c n", p=P
                            ),
                        )
                    nc.scalar.dma_start(
                        out=wkT, in_=wkT_d.rearrange("(c p) e -> p c e", p=P)
                    )
                    for q in range(4):
                        for jp in range(CT // 2):
                            nc.scalar.dma_start(
                                out=ctxT[
                                    :, 2 * jp : 2 * jp + 2, q * 512 : (q + 1) * 512
                                ],
                                in_=ctxT_d[
                                    jp * 2 * P : (jp + 1) * 2 * P,
                                    q * 512 : (q + 1) * 512,
                                ].rearrange("(c p) m -> p c m", p=P),
                            )
                    nc.scalar.dma_start(
                        out=wvT, in_=wvT_d.rearrange("(c p) e -> p c e", p=P)
                    )
                    nc.scalar.dma_start(
                        out=woT, in_=woT_d.rearrange("(c p) e -> p c e", p=P)
                    )

                    # Q projection, contraction-chunk outer: the PE consumes
                    # x chunks as they land
                    qgroups = [
                        (et, chn) for et in range(E // P) for chn in range(N // 512)
                    ]
                    with tc.tile_pool(name="ppsA", bufs=1, space="PSUM") as ppsA:
                        pqs = {
                            g: ppsA.tile([P, 512], F32, tag=f"pq{i}", name=f"pq{i}")
                            for i, g in enumerate(qgroups)
                        }
                        for j in range(CT):
                            for et, chn in qgroups:
                                nc.tensor.matmul(
                                    pqs[(et, chn)],
                                    lhsT=wqT[:, j, et * P : (et + 1) * P],
                                    rhs=xT[:, j, chn * 512 : (chn + 1) * 512],
                                    start=(j == 0),
                                    stop=(j == CT - 1),
                                )
                        for et, chn in qgroups:
                            nc.vector.tensor_copy(
                                qT[:, et, chn * 512 : (chn + 1) * 512],
                                pqs[(et, chn)],
                            )

                def emit_kproj(kps, et, chm):
                    pk = kps.tile([P, 512], F32, tag="pk", name="pk")
                    for j in range(CT):
                        nc.tensor.matmul(
                            pk,
                            lhsT=wkT[:, j, et * P : (et + 1) * P],
                            rhs=ctxT[:, j, chm * 512 : (chm + 1) * 512],
                            start=(j == 0),
                            stop=(j == CT - 1),
                        )
                    nc.vector.tensor_scalar_add(
                        kT[:, et, chm * 512 : (chm + 1) * 512],
                        pk,
                        bk_sb[:, et : et + 1],
                    )

                # stage 1: scores+exp+mask pair A (ACT-bound) with the V and
                # K projections interleaved on the otherwise idle PE. Pair A
                # needs only kT et0; et1 chunks land before stage 2.
                # PSUM: sps1 4 + vps 2 + kps 2 = 8 banks (all double-buffered
                # so no PE stream ever waits on a DVE eviction).
                with (
                    tc.tile_pool(name="sps1", bufs=1, space="PSUM") as sps1,
                    tc.tile_pool(name="vps", bufs=2, space="PSUM") as vps,
                    tc.tile_pool(name="kps", bufs=2, space="PSUM") as kps,
                ):
                    emit_kproj(kps, 0, 0)
                    for mt in range(MT):
                        # keep kT(et0) one chunk ahead of the scores that
                        # consume it; kT(et1) lands before stage 2
                        if mt % 2 == 0:
                            et, chm = divmod(mt // 2 + 1, M // 512)
                            if et < 2:
                                emit_kproj(kps, et, chm)
                        mk = load_mask(mt, nc.gpsimd)
                        for hl in range(2):
                            emit_scores(sps1, 2, hl, mt, exmst, mk)
                        pv = vps.tile([P, HPC, HEAD_DIM], F32, tag="pv")
                        for j in range(CT):
                            nc.tensor.matmul(
                                pv,
                                lhsT=ctxT[:, j, mt * P : (mt + 1) * P],
                                rhs=wvT[:, j, :],
                                start=(j == 0),
                                stop=(j == CT - 1),
                            )
                        # batched strided v eviction (Pool can't read PSUM)
                        nc.vector.tensor_copy(vaug[:, mt, :, :HEAD_DIM], pv)

            # stage 2: PV-A (both n-halves; reads stage-1 exmst, so the PE
            # never waits on a fresh exp) + scores pair B per m-tile. The
            # scores-B masks prefetch on the sync ring, LAG tiles ahead, so
            # they never queue behind normalize shift-DMAs.
            # PSUM: opsA 4 + sps2 4 = 8 banks.
            with (
                tc.tile_pool(name="sps2", bufs=1, space="PSUM") as sps2,
                tc.tile_pool(name="opsA", bufs=1, space="PSUM") as opsA,
            ):
                ot_psA = {
                    chn: [
                        opsA.tile(
                            [HEAD_DIM + 1, 512], F32,
                            tag=f"a{chn}{i}", name=f"a{chn}{i}",
                        )
                        for i in range(2)
                    ]
                    for chn in range(2)
                }
                LAG = 2
                mk_q = [load_mask(mt, nc.sync) for mt in range(LAG)]
                for mt in range(MT):
                    if mt + LAG < MT:
                        mk_q.append(load_mask(mt + LAG, nc.sync))
                    mk = mk_q.pop(0)
                    for chn in range(2):
                        emit_pv(ot_psA[chn], 0, mt, exmst, chn)
                    for hl in range(2):
                        emit_scores(sps2, 2, 2 + hl, mt, exmst, mk)
                # normalize A straight from psum; the DVE/Pool work overlaps
                # stage-3's PV-B sweeps on the PE
                normalize_pair(0, 0, ot_psA[0])
                normalize_pair(0, 1, ot_psA[1])

            # stage 3: PV-B chn0 sweep, then chn1 sweep interleaved with the
            # first half of the output projection; ys evictions + y DMA ride
            # the now-idle ACT engine/ring.
            # PSUM: opsB0 2 + opsB1 2 + ypsum 3 = 7 banks (A drains early).
            with (
                tc.tile_pool(name="opsB0", bufs=1, space="PSUM") as opsB0,
                tc.tile_pool(name="opsB1", bufs=1, space="PSUM") as opsB1,
                tc.tile_pool(name="ypsum", bufs=3, space="PSUM") as ypsum,
                tc.tile_pool(name="ypool", bufs=3) as ypool,
            ):
                def emit_oproj(nb):
                    ys = ypool.tile([P, DIM], F16, tag="ys", name="ys")
                    for oc in range(DIM // 512):
                        yp = ypsum.tile([P, 512], F32, tag="yp", name="yp")
                        for hp in range(E // P):
                            nc.tensor.matmul(
                                yp,
                                lhsT=otn2[:, hp, nb * P : (nb + 1) * P],
                                rhs=woT[:, hp, oc * 512 : (oc + 1) * 512],
                                start=(hp == 0),
                                stop=(hp == E // P - 1),
                            )
                        nc.scalar.copy(ys[:, oc * 512 : (oc + 1) * 512], yp)
                        # per-half writeback, alternating rings: drains the
                        # tail ~2x faster than one whole-row DMA at the end
                        ring = nc.scalar if oc == 0 else nc.sync
                        ring.dma_start(
                            out=y_d[nb * P : (nb + 1) * P, oc * 512 : (oc + 1) * 512],
                            in_=ys[:, oc * 512 : (oc + 1) * 512],
                        )

                ot_psB0 = [
                    opsB0.tile(
                        [HEAD_DIM + 1, 512], F32, tag=f"b0{i}", name=f"b0{i}"
                    )
                    for i in range(2)
                ]
                ot_psB1 = [
                    opsB1.tile(
                        [HEAD_DIM + 1, 512], F32, tag=f"b1{i}", name=f"b1{i}"
                    )
                    for i in range(2)
                ]
                for mt in range(MT):
                    emit_pv(ot_psB0, 1, mt, exmst, 0)
                normalize_pair(1, 0, ot_psB0)
                for mt in range(MT):
                    emit_pv(ot_psB1, 1, mt, exmst, 1)
                    if mt >= 9 and mt % 2 == 1:
                        emit_oproj((mt - 9) // 2)
                normalize_pair(1, 1, ot_psB1)
                for nb in range(4, N // P):
                    emit_oproj(nb)

    nc.compile()
    return nc


_NC_CACHE = []


def _get_nc():
    if not _NC_CACHE:
        _NC_CACHE.append(build_program())
    return _NC_CACHE[0]


def make_in_maps(x, context, mask, Wq, Wkv, b_kv, Wo):
    f16 = ml_dtypes.bfloat16
    f8 = ml_dtypes.float8_e4m3
    x = np.asarray(x, dtype=np.float32)
    context = np.asarray(context, dtype=np.float32)
    mask = np.asarray(mask)
    Wq = np.asarray(Wq, dtype=np.float32)
    Wkv = np.asarray(Wkv, dtype=np.float32)
    b_kv = np.asarray(b_kv, dtype=np.float32)
    Wo = np.asarray(Wo, dtype=np.float32)

    in_maps = []
    for b in range(B):
        xtb = np.ascontiguousarray(x[b].T).astype(f8)
        ctb = np.ascontiguousarray(context[b].T).astype(f16)
        mtb = np.ascontiguousarray(mask[b].T).astype(f16)
        for g in range(NUM_HEADS // HPC):
            sl = slice(E * g, E * (g + 1))
            in_maps.append(
                {
                    "xT": xtb,
                    "ctxT": ctb,
                    "maskt": mtb,
                    "wqT": np.ascontiguousarray(Wq[sl].T).astype(f16),
                    "wkT": np.ascontiguousarray(Wkv[sl].T).astype(f16),
                    "wvT": np.ascontiguousarray(
                        Wkv[DIM + E * g : DIM + E * (g + 1)].T
                    ).astype(f16),
                    "woT": np.ascontiguousarray(Wo[:, sl].T).astype(f16),
                    "bk": np.ascontiguousarray(b_kv[sl]),
                }
            )
    return in_maps


def combine_outputs(ys, b_kv, Wo):
    """ys: list of 8 per-core partial outputs [N, DIM], core order (b, g)."""
    b_v = np.asarray(b_kv, dtype=np.float32)[DIM:]
    ybias = np.asarray(Wo, dtype=np.float32) @ b_v  # [DIM]
    out = np.empty((B, N, DIM), dtype=np.float32)
    G = NUM_HEADS // HPC
    for b in range(B):
        acc = np.asarray(ys[G * b], dtype=np.float32)
        for g in range(1, G):
            acc = acc + np.asarray(ys[G * b + g], dtype=np.float32)
        out[b] = acc + ybias[None, :]
    return out


def kernel(x, context, mask, Wq, Wkv, b_kv, Wo):
    nc = _get_nc()
    in_maps = make_in_maps(x, context, mask, Wq, Wkv, b_kv, Wo)
    res = run_bass_kernel_spmd(nc, in_maps, core_ids=list(range(8)))
    ys = [m["y"] for m in res.results]
    return combine_outputs(ys, b_kv, Wo)
